# revision 18
# baseline (speedup 1.0000x reference)
"""Trainium2 Bass kernel for nn_DriftRectifier (2-block Mamba over 64x64 images).

Sharding: data-parallel over batch B=16 -> 2 samples per core x 8 cores.
v4 architecture (sample-interleaved units + software-pipelined emission):
  - Unit order s0m0, s1m0, s0m1, s1m1: consecutive units are data-
    independent, so unit k+1's proj overlaps unit k's scan.
  - Emission order per k:  S0(k) | T1(k-1) | P(k+1) | T0(k) | S1(k)
    keeps the in-order ACT engine feeding scan exps back-to-back across
    unit boundaries (post/proj ACT work sits in scan-phase slack).
  - Vector (DVE): the 16-n selective scans (tensor_tensor_scan), dbu/hc
    multiplies, gate, LN small ops.
  - Scalar (ACT): exp/silu/softplus(exp+ln batched per half), stats.
    rstd = exp(-0.5*ln(var+eps)) so Ln/Exp stay in ONE table; no Sqrt.
  - GpSimd: post out_proj PSUM->SBUF copy + square (off the ACT critical
    path), output DMA.
  - PE: projections, per-n y accumulation via identity matmuls, LN mean
    and mean-square row-sums in ONE matmul (stacked [x; x^2] rhs), mu+rstd
    [1->64] broadcasts in ONE matmul.
  - DMA: B/C rows partition-broadcast from a DRAM staging tile, one
    descriptor per (n, half) via a 3-dim access pattern.
"""
import contextlib

import numpy as np

B, C, H, W = 16, 4, 64, 64
L = H * W  # 4096
DM, DI, DS, DK, DR = 64, 128, 16, 4, 4
NCORES = 8
BPC = B // NCORES  # samples per core
TC = 512           # psum / matmul chunk
NCH = L // TC      # 8 chunks
HALF = L // 2      # 2048, scan half-sequence
EPS = 1e-5

_CACHE = {}


def _build_program():
    import concourse.bacc as bacc
    import concourse.bass as bass
    from concourse import mybir
    from concourse.tile import TileContext

    F32 = mybir.dt.float32
    BF16 = mybir.dt.bfloat16
    AF = mybir.ActivationFunctionType
    OP = mybir.AluOpType

    nc = bacc.Bacc("TRN2")

    # ---- dram I/O ----
    zc = nc.dram_tensor("zc", [BPC, C, L], F32, kind="ExternalInput")
    out = nc.dram_tensor("out", [BPC, C, L], F32, kind="ExternalOutput")
    ident_in = nc.dram_tensor("ident", [128, 128], BF16, kind="ExternalInput")
    emb_wT = nc.dram_tensor("emb_wT", [C, DM], F32, kind="ExternalInput")
    emb_b = nc.dram_tensor("emb_b", [DM, 1], F32, kind="ExternalInput")
    head_wT = nc.dram_tensor("head_wT", [DM, C], BF16, kind="ExternalInput")
    neg_head_b = nc.dram_tensor("neg_head_b", [C, 1], F32, kind="ExternalInput")
    oh8_in = nc.dram_tensor("oh8", [36, 4 * 128], F32, kind="ExternalInput")
    wsel8_in = nc.dram_tensor("wsel8", [128, 4 * 36], BF16, kind="ExternalInput")
    blk_t = []
    for m in (1, 2):
        p = f"m{m}_"
        blk_t.append({
            "cwu0": nc.dram_tensor(p + "cwu0", [2 * DM, DI], BF16, kind="ExternalInput"),
            "cwu1": nc.dram_tensor(p + "cwu1", [2 * DM, DI], BF16, kind="ExternalInput"),
            "inw_zT": nc.dram_tensor(p + "inw_zT", [DM, DI], BF16, kind="ExternalInput"),
            "conv_b": nc.dram_tensor(p + "conv_b", [DI, 1], F32, kind="ExternalInput"),
            "xpwT": nc.dram_tensor(p + "xpwT", [DI, DR + 2 * DS], BF16, kind="ExternalInput"),
            "dtpwT": nc.dram_tensor(p + "dtpwT", [DR, DI], BF16, kind="ExternalInput"),
            "dtp_b": nc.dram_tensor(p + "dtp_b", [DI, 1], F32, kind="ExternalInput"),
            "A": nc.dram_tensor(p + "A", [DI, DS], F32, kind="ExternalInput"),
            "D": nc.dram_tensor(p + "D", [DI, 1], F32, kind="ExternalInput"),
            "opwT": nc.dram_tensor(p + "opwT", [DI, DM], BF16, kind="ExternalInput"),
            "ln_g": nc.dram_tensor(p + "ln_g", [DM, 1], F32, kind="ExternalInput"),
            "ln_b": nc.dram_tensor(p + "ln_b", [DM, 1], F32, kind="ExternalInput"),
        })

    with TileContext(nc) as tc, contextlib.ExitStack() as ctx:
        consts = ctx.enter_context(tc.tile_pool(name="consts", bufs=1))
        persist = ctx.enter_context(tc.tile_pool(name="persist", bufs=1))
        bcw = ctx.enter_context(tc.tile_pool(name="bcw", bufs=3))
        enw = ctx.enter_context(tc.tile_pool(name="enw", bufs=3))
        nwork = ctx.enter_context(tc.tile_pool(name="nwork", bufs=2))
        small = ctx.enter_context(tc.tile_pool(name="small", bufs=2))
        stp = ctx.enter_context(tc.tile_pool(name="stp", bufs=1))
        postw = ctx.enter_context(tc.tile_pool(name="postw", bufs=2))
        psA = ctx.enter_context(tc.tile_pool(name="psA", bufs=2, space="PSUM"))
        psB = ctx.enter_context(tc.tile_pool(name="psB", bufs=2, space="PSUM"))
        psY = ctx.enter_context(tc.tile_pool(name="psY", bufs=1, space="PSUM"))
        dstage = ctx.enter_context(tc.tile_pool(name="dstage", bufs=4, space="DRAM"))

        # ---- constants to SBUF ----
        ident = consts.tile([128, 128], BF16)
        nc.sync.dma_start(out=ident, in_=ident_in[:])
        sb_embT = consts.tile([C, DM], F32)
        nc.sync.dma_start(out=sb_embT, in_=emb_wT[:])
        sb_embb = consts.tile([DM, 1], F32)
        nc.sync.dma_start(out=sb_embb, in_=emb_b[:])
        sb_headT = consts.tile([DM, C], BF16)
        nc.sync.dma_start(out=sb_headT, in_=head_wT[:])
        sb_nhb = consts.tile([C, 1], F32)
        nc.sync.dma_start(out=sb_nhb, in_=neg_head_b[:])
        eps8 = consts.tile([NCH, 1], F32)
        nc.vector.memset(eps8, EPS)
        oh8 = consts.tile([36, 4 * 128], F32)
        nc.sync.dma_start(out=oh8, in_=oh8_in[:])
        wsel8 = consts.tile([128, 4 * 36], BF16)
        nc.sync.dma_start(out=wsel8, in_=wsel8_in[:])
        blk = []
        for m in range(2):
            d = {}
            for k, t in blk_t[m].items():
                d[k] = consts.tile(list(t.shape), t.dtype, name=f"c_m{m}_{k}")
                nc.sync.dma_start(out=d[k], in_=t[:])
            blk.append(d)

        # ---- persistent tiles ----
        # feat2x: PER-SAMPLE (embed / post-LN writes, next block's proj reads)
        feat2x = [persist.tile([2 * DM, L + 3], BF16, name=f"feat2x{i}")
                  for i in range(2)]
        # parity-duplicated so unit k+1's proj can overlap unit k's scan/post
        u_bf = [persist.tile([DI, L], BF16, name=f"u{i}") for i in range(2)]
        zs_bf = [persist.tile([DI, L], BF16, name=f"zs{i}") for i in range(2)]
        dtu_bf = [persist.tile([DI, L], BF16, name=f"dtu{i}") for i in range(2)]
        dt_f32 = [persist.tile([DI, L], BF16, name=f"dt{i}") for i in range(2)]
        carry = [persist.tile([DI, DS], F32, name=f"carry{i}") for i in range(2)]

        def emit_embed(si, crange=None):
            with nc.named_scope(f"s{si}_embed"):
                for c in (crange if crange is not None else range(NCH)):
                    cs = slice(c * TC, (c + 1) * TC)
                    zch = small.tile([C, TC], F32, name="zch", tag="zch")
                    nc.scalar.dma_start(out=zch, in_=zc[si][:, cs])
                    ps = psA.tile([DM, TC], F32, name="emb_ps", tag="mm")
                    nc.tensor.matmul(ps, lhsT=sb_embT, rhs=zch,
                                     start=True, stop=True)
                    nc.scalar.activation(
                        out=feat2x[si][0:DM, 3 + c * TC:3 + (c + 1) * TC],
                        in_=ps, func=AF.Identity, bias=sb_embb[:, :])
                    nc.scalar.activation(
                        out=feat2x[si][DM:2 * DM, 2 + c * TC:2 + (c + 1) * TC],
                        in_=ps, func=AF.Identity, bias=sb_embb[:, :])
                if 0 in (crange or range(NCH)):
                    nc.vector.memset(feat2x[si][0:DM, 0:3], 0.0)
                    nc.vector.memset(feat2x[si][DM:2 * DM, 0:2], 0.0)

        # n-values whose scan chain runs on GpSimd (second scan engine)
        POOL_NS = frozenset()
        # sample-interleaved unit order: consecutive units are independent
        units = [(0, 0), (1, 0), (0, 1), (1, 1)]
        NU = len(units)
        UST = [{} for _ in range(NU)]

        def proj_begin(ui):
            UST[ui]["bc"] = dstage.tile([2 * DS, L], BF16, name="bc_dram")

        def proj_silu(ui, crange):
            s, m = units[ui]
            w = blk[m]
            par = ui % 2
            u_t, zs_t = u_bf[par], zs_bf[par]
            f2x = feat2x[s]
            with nc.named_scope(f"s{s}m{m}_proj"):
                # silu pass (conv fused into in_proj via shifted feat2x)
                for c in crange:
                    cs = slice(c * TC, (c + 1) * TC)
                    ups = psA.tile([DI, TC], F32, name="ups", tag="mm")
                    nc.tensor.matmul(ups, lhsT=w["cwu0"],
                                     rhs=f2x[:, c * TC:c * TC + TC],
                                     start=True, stop=False)
                    nc.tensor.matmul(ups, lhsT=w["cwu1"],
                                     rhs=f2x[:, c * TC + 2:c * TC + 2 + TC],
                                     start=False, stop=True)
                    nc.scalar.activation(out=u_t[:, cs], in_=ups, func=AF.Silu,
                                         bias=w["conv_b"][:, :])
                    zps = psA.tile([DI, TC], F32, name="zps", tag="mm")
                    nc.tensor.matmul(zps, lhsT=w["inw_zT"],
                                     rhs=f2x[0:DM, 3 + c * TC:3 + (c + 1) * TC],
                                     start=True, stop=True)
                    nc.scalar.activation(out=zs_t[:, cs], in_=zps, func=AF.Silu)

        def proj_xp(ui, half):
            s, m = units[ui]
            w = blk[m]
            par = ui % 2
            u_t, dt_t = u_bf[par], dt_f32[par]
            dtu_t = dtu_bf[par]
            bc_dram = UST[ui]["bc"]
            # x_proj / dt pass: exps batched per half, then one ln
            with nc.named_scope(f"s{s}m{m}_proj"):
                if True:
                    spe = enw.tile([DI, HALF], F32, name="spe", tag="en")
                    for cc in range(NCH // 2):
                        c = half * (NCH // 2) + cc
                        cs = slice(c * TC, (c + 1) * TC)
                        xps = psA.tile([DR + 2 * DS, TC], F32, name="xps", tag="mm")
                        nc.tensor.matmul(xps, lhsT=w["xpwT"], rhs=u_t[:, cs],
                                         start=True, stop=True)
                        # x_proj rows host-permuted to [B(16), C(16), dt(4)]
                        bcc = small.tile([2 * DS, TC], BF16, name="bcc", tag="bcc")
                        nc.scalar.activation(out=bcc, in_=xps[0:2 * DS, :],
                                             func=AF.Copy)
                        nc.sync.dma_start(out=bc_dram[:, cs], in_=bcc)
                        dtr = small.tile([DR, TC], BF16, name="dtr", tag="dtr")
                        nc.scalar.activation(out=dtr,
                                             in_=xps[2 * DS:2 * DS + DR, :],
                                             func=AF.Copy)
                        dtps = psA.tile([DI, TC], F32, name="dtps", tag="mm")
                        nc.tensor.matmul(dtps, lhsT=w["dtpwT"], rhs=dtr,
                                         start=True, stop=True)
                        # softplus(x) = ln(1 + exp(x))
                        nc.scalar.activation(out=spe[:, cc * TC:(cc + 1) * TC],
                                             in_=dtps, func=AF.Exp,
                                             bias=w["dtp_b"][:, :])
                    hsl = slice(half * HALF, (half + 1) * HALF)
                    nc.scalar.activation(out=dt_t[:, hsl],
                                         in_=spe, func=AF.Ln, bias=1.0)
                    for j in range(2):
                        qj = slice(half * HALF + j * (HALF // 2),
                                   half * HALF + (j + 1) * (HALF // 2))
                        nc.vector.tensor_tensor(out=dtu_t[:, qj], in0=dt_t[:, qj],
                                                in1=u_t[:, qj], op=OP.mult)

        def proj_full(ui):
            proj_begin(ui)
            proj_silu(ui, range(NCH))
            proj_xp(ui, 0)
            proj_xp(ui, 1)

        def scan_half(ui, q):
            s, m = units[ui]
            w = blk[m]
            par = ui % 2
            u_t, zs_t, dtu_t, dt_t = u_bf[par], zs_bf[par], dtu_bf[par], dt_f32[par]
            carry_t = carry[par]
            bc_dram = UST[ui]["bc"]
            with nc.named_scope(f"s{s}m{m}_scan{q}"):
                hs = q * HALF
                qsl = slice(hs, hs + HALF)
                yo_h = postw.tile([DI, HALF], BF16, name="yo_h", tag="yo")
                UST[ui][f"yo{q}"] = yo_h
                yps = [psY.tile([DI, TC], F32, name=f"yps{k}", tag=f"yps{k}")
                       for k in range(HALF // TC)]
                for n in range(DS):
                    veng = nc.gpsimd if n in POOL_NS else nc.vector
                    en = enw.tile([DI, HALF], F32, name="en", tag="en")
                    nc.scalar.activation(out=en, in_=dt_t[:, qsl],
                                         func=AF.Exp,
                                         scale=w["A"][:, n:n + 1])
                    bc_t = bcw.tile([DI, 2 * HALF], BF16, name="bc_t",
                                    tag="bc_t")
                    nc.sync.dma_start(out=bc_t, in_=bass.AP(
                        tensor=bc_dram.tensor,
                        offset=bc_dram.offset + n * L + hs,
                        ap=[[0, DI], [DS * L, 2], [1, HALF]]))
                    dbu = nwork.tile([DI, HALF], BF16, name="dbu", tag="dbu")
                    veng.tensor_tensor(out=dbu, in0=dtu_t[:, qsl],
                                       in1=bc_t[:, 0:HALF], op=OP.mult)
                    h_t = nwork.tile([DI, HALF], BF16, name="h_t", tag="h_t")
                    init = 0.0 if q == 0 else carry_t[:, n:n + 1]
                    veng.tensor_tensor_scan(
                        out=h_t, data0=en, data1=dbu,
                        initial=init, op0=OP.mult, op1=OP.add)
                    if q == 0:
                        veng.tensor_copy(out=carry_t[:, n:n + 1],
                                         in_=h_t[:, HALF - 1:HALF])
                    hc = nwork.tile([DI, HALF], BF16, name="hc", tag="hc")
                    veng.tensor_tensor(out=hc, in0=h_t,
                                       in1=bc_t[:, HALF:2 * HALF],
                                       op=OP.mult)
                    for k in range(HALF // TC):
                        nc.tensor.matmul(yps[k], lhsT=ident,
                                         rhs=hc[:, k * TC:(k + 1) * TC],
                                         start=(n == 0), stop=(n == DS - 1))
                for k in range(HALF // TC):
                    cs = slice(hs + k * TC, hs + (k + 1) * TC)
                    ks = slice(k * TC, (k + 1) * TC)
                    tmp = small.tile([DI, TC], BF16, name="ytmp", tag="ytmp")
                    nc.vector.scalar_tensor_tensor(
                        out=tmp, in0=u_t[:, cs], scalar=w["D"][:, :],
                        in1=yps[k], op0=OP.mult, op1=OP.add)
                    nc.gpsimd.tensor_tensor(out=yo_h[:, ks], in0=tmp,
                                            in1=zs_t[:, cs], op=OP.mult)

        def post_half(ui, h2):
            s, m = units[ui]
            w = blk[m]
            f2x = feat2x[s]
            yo_h = UST[ui][f"yo{h2}"]
            with nc.named_scope(f"s{s}m{m}_post{h2}"):
                fchsq = postw.tile([2 * DM, HALF], BF16, name="fchsq",
                                   tag="fchsq")
                # mean rows 0:4, mean-square rows 32:36 (PSUM reads must
                # start at a 32-partition boundary)
                ps8 = psB.tile([36, TC], F32, name="ps8", tag="ps8")
                for cc in range(4):
                    ls = slice(cc * TC, (cc + 1) * TC)
                    fps = psA.tile([DM, TC], F32, name="fps", tag="mm")
                    nc.tensor.matmul(fps, lhsT=w["opwT"], rhs=yo_h[:, ls],
                                     start=True, stop=True)
                    # square on GpSimd (from the SBUF copy; GpSimd cannot
                    # read PSUM): halves the post load on the in-order ACT
                    nc.scalar.activation(out=fchsq[0:DM, ls], in_=fps,
                                         func=AF.Copy)
                    nc.gpsimd.tensor_tensor(out=fchsq[DM:2 * DM, ls],
                                            in0=fchsq[0:DM, ls],
                                            in1=fchsq[0:DM, ls], op=OP.mult)
                    nc.tensor.matmul(ps8, lhsT=wsel8[:, cc * 36:(cc + 1) * 36],
                                     rhs=fchsq[:, ls],
                                     start=(cc == 0), stop=(cc == 3))
                # stats rows 0:4 = mu, rows 32:36 = rstd
                stats = postw.tile([36, TC], F32, name="stats", tag="st8")
                nc.scalar.activation(out=stats[0:4, :], in_=ps8[0:4, :],
                                     func=AF.Copy)
                mu2 = stp.tile([4, TC], F32, name="mu2", tag="mu2")
                nc.vector.tensor_tensor(out=mu2, in0=stats[0:4, :],
                                        in1=stats[0:4, :], op=OP.mult)
                var4 = stp.tile([4, TC], F32, name="var4", tag="var")
                nc.vector.tensor_tensor(out=var4, in0=ps8[32:36, :], in1=mu2,
                                        op=OP.subtract)
                # rstd = (var+eps)^-1/2 = exp(-0.5*ln(var+eps)); Ln and Exp
                # coexist in one ACT table (no Sqrt table switch)
                lnv = stp.tile([4, TC], F32, name="lnv", tag="lnv")
                nc.scalar.activation(out=lnv, in_=var4, func=AF.Ln,
                                     bias=eps8[0:4, :])
                nc.scalar.activation(out=stats[32:36, :], in_=lnv,
                                     func=AF.Exp, scale=-0.5)
                for cc in range(4):
                    c = h2 * 4 + cc
                    cs = slice(c * TC, (c + 1) * TC)
                    ls = slice(cc * TC, (cc + 1) * TC)
                    mrbc = psA.tile([2 * DM, TC], F32, name="mrbc", tag="mm")
                    nc.tensor.matmul(mrbc,
                                     lhsT=oh8[:, cc * 128:(cc + 1) * 128],
                                     rhs=stats, start=True, stop=True)
                    t1 = small.tile([DM, TC], BF16, name="t1", tag="t1")
                    nc.vector.tensor_tensor(out=t1, in0=fchsq[0:DM, ls],
                                            in1=mrbc[0:DM, :], op=OP.subtract)
                    t2 = small.tile([DM, TC], BF16, name="t2", tag="t2")
                    nc.vector.tensor_tensor(out=t2, in0=t1,
                                            in1=mrbc[DM:2 * DM, :], op=OP.mult)
                    if m == 0:
                        nc.scalar.activation(
                            out=f2x[0:DM, 3 + c * TC:3 + (c + 1) * TC],
                            in_=t2, func=AF.Identity,
                            scale=w["ln_g"][:, :], bias=w["ln_b"][:, :])
                        nc.scalar.activation(
                            out=f2x[DM:2 * DM, 2 + c * TC:2 + (c + 1) * TC],
                            in_=t2, func=AF.Identity,
                            scale=w["ln_g"][:, :], bias=w["ln_b"][:, :])
                    else:
                        # head input staged in a temp tile so the next
                        # sample's embed never waits on feat2x
                        hd = small.tile([DM, TC], BF16, name="hd", tag="hd")
                        nc.scalar.activation(
                            out=hd, in_=t2, func=AF.Identity,
                            scale=w["ln_g"][:, :], bias=w["ln_b"][:, :])
                        dps = psA.tile([C, TC], F32, name="dps", tag="mm")
                        nc.tensor.matmul(dps, lhsT=sb_headT, rhs=hd,
                                         start=True, stop=True)
                        nd = small.tile([C, TC], BF16, name="nd", tag="nd")
                        nc.scalar.activation(out=nd, in_=dps,
                                             func=AF.Identity,
                                             scale=-1.0, bias=sb_nhb[:, :])
                        zch2 = small.tile([C, TC], F32, name="zch2",
                                          tag="zch")
                        nc.scalar.dma_start(out=zch2, in_=zc[s][:, cs])
                        oc = small.tile([C, TC], F32, name="oc", tag="zch")
                        nc.gpsimd.tensor_tensor(out=oc, in0=zch2, in1=nd,
                                                op=OP.add)
                        nc.gpsimd.dma_start(out=out[s][:, cs], in_=oc)

        # ---- software-pipelined emission ----
        # unit 0 startup split by half so the first scan starts sooner
        proj_begin(0)
        emit_embed(0, range(0, NCH // 2))
        proj_silu(0, range(0, NCH // 2))
        proj_xp(0, 0)
        for k in range(NU):
            scan_half(k, 0)
            if k == 0:
                emit_embed(0, range(NCH // 2, NCH))
                proj_silu(0, range(NCH // 2, NCH))
                proj_xp(0, 1)
                emit_embed(1)
            if k > 0:
                post_half(k - 1, 1)
            if k + 1 < NU:
                proj_full(k + 1)
            post_half(k, 0)
            scan_half(k, 1)
        post_half(NU - 1, 1)

    nc.finalize()
    return nc


def _prep_maps(inputs):
    import ml_dtypes
    bf = ml_dtypes.bfloat16
    f = np.float32
    z = np.asarray(inputs["z_damaged"], dtype=f).reshape(B, C, L)

    # oh8: per-cc [36,128] broadcast weights: out rows 0:64 get mu[cc],
    # rows 64:128 get rstd[cc]  (stats rows 0:4 = mu, 32:36 = rstd)
    oh8 = np.zeros((36, 4 * 128), f)
    for cc in range(4):
        oh8[cc, cc * 128:cc * 128 + 64] = 1.0
        oh8[32 + cc, cc * 128 + 64:cc * 128 + 128] = 1.0
    # wsel8: per-cc [128,36]: col cc sums rows 0:64 (x)/64, col 32+cc
    # sums rows 64:128 (x^2)/64 (msq lands at PSUM partition 32)
    wsel8 = np.zeros((128, 4 * 36), f)
    for cc in range(4):
        wsel8[0:64, cc * 36 + cc] = 1.0 / DM
        wsel8[64:128, cc * 36 + 32 + cc] = 1.0 / DM

    base = {
        "ident": np.eye(128, dtype=bf),
        "emb_wT": np.ascontiguousarray(np.asarray(inputs["emb_w"], f).T),
        "emb_b": np.asarray(inputs["emb_b"], f).reshape(DM, 1),
        "head_wT": np.ascontiguousarray(np.asarray(inputs["head_w"], f).T).astype(bf),
        "neg_head_b": (-np.asarray(inputs["head_b"], f)).reshape(C, 1),
        "oh8": oh8,
        "wsel8": wsel8.astype(bf),
    }
    for m in (1, 2):
        p = f"m{m}_"
        inw = np.asarray(inputs[p + "in_proj_w"], f)  # [2DI, DM]
        w_u = inw[:DI]  # [DI, DM]
        cw = np.asarray(inputs[p + "conv_w"], f).reshape(DI, DK)
        # lhsT rows (k,m) -> cols d: w[d,k]*W_u[d,m]
        base[p + "cwu0"] = np.ascontiguousarray(np.concatenate(
            [cw[:, 0][None, :] * w_u.T, cw[:, 1][None, :] * w_u.T], axis=0)).astype(bf)
        base[p + "cwu1"] = np.ascontiguousarray(np.concatenate(
            [cw[:, 2][None, :] * w_u.T, cw[:, 3][None, :] * w_u.T], axis=0)).astype(bf)
        base[p + "inw_zT"] = np.ascontiguousarray(inw[DI:].T).astype(bf)
        base[p + "conv_b"] = np.asarray(inputs[p + "conv_b"], f).reshape(DI, 1)
        xpw = np.asarray(inputs[p + "x_proj_w"], f)  # rows: dt(4), B(16), C(16)
        xpw = np.concatenate([xpw[DR:], xpw[:DR]], axis=0)  # -> B, C, dt
        base[p + "xpwT"] = np.ascontiguousarray(xpw.T).astype(bf)
        base[p + "dtpwT"] = np.ascontiguousarray(
            np.asarray(inputs[p + "dt_proj_w"], f).T).astype(bf)
        base[p + "dtp_b"] = np.asarray(inputs[p + "dt_proj_b"], f).reshape(DI, 1)
        base[p + "A"] = -np.exp(np.asarray(inputs[p + "A_log"], f))
        base[p + "D"] = np.asarray(inputs[p + "D"], f).reshape(DI, 1)
        base[p + "opwT"] = np.ascontiguousarray(
            np.asarray(inputs[p + "out_proj_w"], f).T).astype(bf)
        base[p + "ln_g"] = np.asarray(inputs[f"ln{m}_g"], f).reshape(DM, 1)
        base[p + "ln_b"] = np.asarray(inputs[f"ln{m}_b"], f).reshape(DM, 1)

    maps = []
    for k in range(NCORES):
        mkp = dict(base)
        mkp["zc"] = np.ascontiguousarray(z[k * BPC:(k + 1) * BPC])
        maps.append(mkp)
    return maps


def _run(inputs, trace=False):
    from concourse.bass_utils import run_bass_kernel_spmd
    if "nc" not in _CACHE:
        _CACHE["nc"] = _build_program()
    nc = _CACHE["nc"]
    maps = _prep_maps(inputs)
    res = run_bass_kernel_spmd(nc, maps, core_ids=list(range(NCORES)), trace=trace)
    outs = [r["out"] for r in res.results]
    full = np.concatenate(outs, axis=0).reshape(B, C, H, W)
    return full, res


def kernel(**inputs):
    full, _ = _run(inputs, trace=False)
    return full


# revision 19
# speedup vs baseline: 1.0039x; 1.0039x over previous
"""Trainium2 Bass kernel for nn_DriftRectifier (2-block Mamba over 64x64 images).

Sharding: data-parallel over batch B=16 -> 2 samples per core x 8 cores.
v4 architecture (sample-interleaved units + software-pipelined emission):
  - Unit order s0m0, s1m0, s0m1, s1m1: consecutive units are data-
    independent, so unit k+1's proj overlaps unit k's scan.
  - Emission order per k:  S0(k) | T1(k-1) | P(k+1) | T0(k) | S1(k)
    keeps the in-order ACT engine feeding scan exps back-to-back across
    unit boundaries (post/proj ACT work sits in scan-phase slack).
  - Vector (DVE): the 16-n selective scans (tensor_tensor_scan), dbu/hc
    multiplies, gate, LN small ops.
  - Scalar (ACT): exp/silu/softplus(exp+ln batched per half), stats.
    rstd = exp(-0.5*ln(var+eps)) so Ln/Exp stay in ONE table; no Sqrt.
  - GpSimd: post out_proj PSUM->SBUF copy + square (off the ACT critical
    path), output DMA.
  - PE: projections, per-n y accumulation via identity matmuls, LN mean
    and mean-square row-sums in ONE matmul (stacked [x; x^2] rhs), mu+rstd
    [1->64] broadcasts in ONE matmul.
  - DMA: B/C rows partition-broadcast from a DRAM staging tile, one
    descriptor per (n, half) via a 3-dim access pattern.
"""
import contextlib

import numpy as np

B, C, H, W = 16, 4, 64, 64
L = H * W  # 4096
DM, DI, DS, DK, DR = 64, 128, 16, 4, 4
NCORES = 8
BPC = B // NCORES  # samples per core
TC = 512           # psum / matmul chunk
NCH = L // TC      # 8 chunks
HALF = L // 2      # 2048, scan half-sequence
EPS = 1e-5

_CACHE = {}


def _build_program():
    import concourse.bacc as bacc
    import concourse.bass as bass
    from concourse import mybir
    from concourse.tile import TileContext

    F32 = mybir.dt.float32
    BF16 = mybir.dt.bfloat16
    AF = mybir.ActivationFunctionType
    OP = mybir.AluOpType

    nc = bacc.Bacc("TRN2")

    # ---- dram I/O ----
    zc = nc.dram_tensor("zc", [BPC, C, L], F32, kind="ExternalInput")
    out = nc.dram_tensor("out", [BPC, C, L], F32, kind="ExternalOutput")
    ident_in = nc.dram_tensor("ident", [128, 128], BF16, kind="ExternalInput")
    emb_wT = nc.dram_tensor("emb_wT", [C, DM], F32, kind="ExternalInput")
    emb_b = nc.dram_tensor("emb_b", [DM, 1], F32, kind="ExternalInput")
    head_wT = nc.dram_tensor("head_wT", [DM, C], BF16, kind="ExternalInput")
    neg_head_b = nc.dram_tensor("neg_head_b", [C, 1], F32, kind="ExternalInput")
    oh8_in = nc.dram_tensor("oh8", [36, 4 * 128], F32, kind="ExternalInput")
    wsel8_in = nc.dram_tensor("wsel8", [128, 4 * 36], BF16, kind="ExternalInput")
    blk_t = []
    for m in (1, 2):
        p = f"m{m}_"
        blk_t.append({
            "cwu0": nc.dram_tensor(p + "cwu0", [2 * DM, DI], BF16, kind="ExternalInput"),
            "cwu1": nc.dram_tensor(p + "cwu1", [2 * DM, DI], BF16, kind="ExternalInput"),
            "inw_zT": nc.dram_tensor(p + "inw_zT", [DM, DI], BF16, kind="ExternalInput"),
            "conv_b": nc.dram_tensor(p + "conv_b", [DI, 1], F32, kind="ExternalInput"),
            "xpwT": nc.dram_tensor(p + "xpwT", [DI, DR + 2 * DS], BF16, kind="ExternalInput"),
            "dtpwT": nc.dram_tensor(p + "dtpwT", [DR, DI], BF16, kind="ExternalInput"),
            "dtp_b": nc.dram_tensor(p + "dtp_b", [DI, 1], F32, kind="ExternalInput"),
            "A": nc.dram_tensor(p + "A", [DI, DS], F32, kind="ExternalInput"),
            "D": nc.dram_tensor(p + "D", [DI, 1], F32, kind="ExternalInput"),
            "opwT": nc.dram_tensor(p + "opwT", [DI, DM], BF16, kind="ExternalInput"),
            "ln_g": nc.dram_tensor(p + "ln_g", [DM, 1], F32, kind="ExternalInput"),
            "ln_b": nc.dram_tensor(p + "ln_b", [DM, 1], F32, kind="ExternalInput"),
        })

    with TileContext(nc) as tc, contextlib.ExitStack() as ctx:
        consts = ctx.enter_context(tc.tile_pool(name="consts", bufs=1))
        persist = ctx.enter_context(tc.tile_pool(name="persist", bufs=1))
        bcw = ctx.enter_context(tc.tile_pool(name="bcw", bufs=3))
        enw = ctx.enter_context(tc.tile_pool(name="enw", bufs=3))
        nwork = ctx.enter_context(tc.tile_pool(name="nwork", bufs=2))
        small = ctx.enter_context(tc.tile_pool(name="small", bufs=2))
        stp = ctx.enter_context(tc.tile_pool(name="stp", bufs=1))
        postw = ctx.enter_context(tc.tile_pool(name="postw", bufs=2))
        psA = ctx.enter_context(tc.tile_pool(name="psA", bufs=2, space="PSUM"))
        psB = ctx.enter_context(tc.tile_pool(name="psB", bufs=2, space="PSUM"))
        psY = ctx.enter_context(tc.tile_pool(name="psY", bufs=1, space="PSUM"))
        dstage = ctx.enter_context(tc.tile_pool(name="dstage", bufs=4, space="DRAM"))

        # ---- constants to SBUF ----
        ident = consts.tile([128, 128], BF16)
        nc.sync.dma_start(out=ident, in_=ident_in[:])
        sb_embT = consts.tile([C, DM], F32)
        nc.sync.dma_start(out=sb_embT, in_=emb_wT[:])
        sb_embb = consts.tile([DM, 1], F32)
        nc.sync.dma_start(out=sb_embb, in_=emb_b[:])
        sb_headT = consts.tile([DM, C], BF16)
        nc.sync.dma_start(out=sb_headT, in_=head_wT[:])
        sb_nhb = consts.tile([C, 1], F32)
        nc.sync.dma_start(out=sb_nhb, in_=neg_head_b[:])
        eps8 = consts.tile([NCH, 1], F32)
        nc.vector.memset(eps8, EPS)
        oh8 = consts.tile([36, 4 * 128], F32)
        nc.sync.dma_start(out=oh8, in_=oh8_in[:])
        wsel8 = consts.tile([128, 4 * 36], BF16)
        nc.sync.dma_start(out=wsel8, in_=wsel8_in[:])
        blk = []
        for m in range(2):
            d = {}
            for k, t in blk_t[m].items():
                d[k] = consts.tile(list(t.shape), t.dtype, name=f"c_m{m}_{k}")
                nc.sync.dma_start(out=d[k], in_=t[:])
            blk.append(d)

        # ---- persistent tiles ----
        # feat2x: PER-SAMPLE (embed / post-LN writes, next block's proj reads)
        feat2x = [persist.tile([2 * DM, L + 3], BF16, name=f"feat2x{i}")
                  for i in range(2)]
        # parity-duplicated so unit k+1's proj can overlap unit k's scan/post
        u_bf = [persist.tile([DI, L], BF16, name=f"u{i}") for i in range(2)]
        zs_bf = [persist.tile([DI, L], BF16, name=f"zs{i}") for i in range(2)]
        dtu_bf = [persist.tile([DI, L], BF16, name=f"dtu{i}") for i in range(2)]
        dt_f32 = [persist.tile([DI, L], BF16, name=f"dt{i}") for i in range(2)]
        carry = [persist.tile([DI, DS], F32, name=f"carry{i}") for i in range(2)]

        def emit_embed(si, crange=None):
            with nc.named_scope(f"s{si}_embed"):
                for c in (crange if crange is not None else range(NCH)):
                    cs = slice(c * TC, (c + 1) * TC)
                    zch = small.tile([C, TC], F32, name="zch", tag="zch")
                    nc.scalar.dma_start(out=zch, in_=zc[si][:, cs])
                    ps = psA.tile([DM, TC], F32, name="emb_ps", tag="mm")
                    nc.tensor.matmul(ps, lhsT=sb_embT, rhs=zch,
                                     start=True, stop=True)
                    nc.scalar.activation(
                        out=feat2x[si][0:DM, 3 + c * TC:3 + (c + 1) * TC],
                        in_=ps, func=AF.Identity, bias=sb_embb[:, :])
                    nc.scalar.activation(
                        out=feat2x[si][DM:2 * DM, 2 + c * TC:2 + (c + 1) * TC],
                        in_=ps, func=AF.Identity, bias=sb_embb[:, :])
                if 0 in (crange or range(NCH)):
                    nc.vector.memset(feat2x[si][0:DM, 0:3], 0.0)
                    nc.vector.memset(feat2x[si][DM:2 * DM, 0:2], 0.0)

        # n-values whose scan chain runs on GpSimd (second scan engine)
        POOL_NS = frozenset()
        # sample-interleaved unit order: consecutive units are independent
        units = [(0, 0), (1, 0), (0, 1), (1, 1)]
        NU = len(units)
        UST = [{} for _ in range(NU)]

        def proj_begin(ui):
            UST[ui]["bc"] = dstage.tile([2 * DS, L], BF16, name="bc_dram")

        def proj_silu(ui, crange):
            s, m = units[ui]
            w = blk[m]
            par = ui % 2
            u_t, zs_t = u_bf[par], zs_bf[par]
            f2x = feat2x[s]
            with nc.named_scope(f"s{s}m{m}_proj"):
                # silu pass (conv fused into in_proj via shifted feat2x)
                for c in crange:
                    cs = slice(c * TC, (c + 1) * TC)
                    ups = psA.tile([DI, TC], F32, name="ups", tag="mm")
                    nc.tensor.matmul(ups, lhsT=w["cwu0"],
                                     rhs=f2x[:, c * TC:c * TC + TC],
                                     start=True, stop=False)
                    nc.tensor.matmul(ups, lhsT=w["cwu1"],
                                     rhs=f2x[:, c * TC + 2:c * TC + 2 + TC],
                                     start=False, stop=True)
                    nc.scalar.activation(out=u_t[:, cs], in_=ups, func=AF.Silu,
                                         bias=w["conv_b"][:, :])
                    zps = psA.tile([DI, TC], F32, name="zps", tag="mm")
                    nc.tensor.matmul(zps, lhsT=w["inw_zT"],
                                     rhs=f2x[0:DM, 3 + c * TC:3 + (c + 1) * TC],
                                     start=True, stop=True)
                    nc.scalar.activation(out=zs_t[:, cs], in_=zps, func=AF.Silu)

        def proj_xp(ui, half):
            s, m = units[ui]
            w = blk[m]
            par = ui % 2
            u_t, dt_t = u_bf[par], dt_f32[par]
            dtu_t = dtu_bf[par]
            bc_dram = UST[ui]["bc"]
            # x_proj / dt pass: exps batched per half, then one ln
            with nc.named_scope(f"s{s}m{m}_proj"):
                if True:
                    spe = enw.tile([DI, HALF], F32, name="spe", tag="en")
                    for cc in range(NCH // 2):
                        c = half * (NCH // 2) + cc
                        cs = slice(c * TC, (c + 1) * TC)
                        xps = psA.tile([DR + 2 * DS, TC], F32, name="xps", tag="mm")
                        nc.tensor.matmul(xps, lhsT=w["xpwT"], rhs=u_t[:, cs],
                                         start=True, stop=True)
                        # x_proj rows host-permuted to [B(16), C(16), dt(4)]
                        bcc = small.tile([2 * DS, TC], BF16, name="bcc", tag="bcc")
                        nc.scalar.activation(out=bcc, in_=xps[0:2 * DS, :],
                                             func=AF.Copy)
                        nc.sync.dma_start(out=bc_dram[:, cs], in_=bcc)
                        dtr = small.tile([DR, TC], BF16, name="dtr", tag="dtr")
                        nc.scalar.activation(out=dtr,
                                             in_=xps[2 * DS:2 * DS + DR, :],
                                             func=AF.Copy)
                        dtps = psA.tile([DI, TC], F32, name="dtps", tag="mm")
                        nc.tensor.matmul(dtps, lhsT=w["dtpwT"], rhs=dtr,
                                         start=True, stop=True)
                        # softplus(x) = ln(1 + exp(x))
                        nc.scalar.activation(out=spe[:, cc * TC:(cc + 1) * TC],
                                             in_=dtps, func=AF.Exp,
                                             bias=w["dtp_b"][:, :])
                    hsl = slice(half * HALF, (half + 1) * HALF)
                    nc.scalar.activation(out=dt_t[:, hsl],
                                         in_=spe, func=AF.Ln, bias=1.0)
                    for j in range(2):
                        qj = slice(half * HALF + j * (HALF // 2),
                                   half * HALF + (j + 1) * (HALF // 2))
                        nc.vector.tensor_tensor(out=dtu_t[:, qj], in0=dt_t[:, qj],
                                                in1=u_t[:, qj], op=OP.mult)

        def proj_full(ui):
            proj_begin(ui)
            proj_silu(ui, range(NCH))
            proj_xp(ui, 0)
            proj_xp(ui, 1)

        def scan_half(ui, q):
            s, m = units[ui]
            w = blk[m]
            par = ui % 2
            u_t, zs_t, dtu_t, dt_t = u_bf[par], zs_bf[par], dtu_bf[par], dt_f32[par]
            carry_t = carry[par]
            bc_dram = UST[ui]["bc"]
            with nc.named_scope(f"s{s}m{m}_scan{q}"):
                hs = q * HALF
                qsl = slice(hs, hs + HALF)
                yo_h = postw.tile([DI, HALF], BF16, name="yo_h", tag="yo")
                UST[ui][f"yo{q}"] = yo_h
                yps = [psY.tile([DI, TC], F32, name=f"yps{k}", tag=f"yps{k}")
                       for k in range(HALF // TC)]
                for n in range(DS):
                    veng = nc.gpsimd if n in POOL_NS else nc.vector
                    en = enw.tile([DI, HALF], F32, name="en", tag="en")
                    nc.scalar.activation(out=en, in_=dt_t[:, qsl],
                                         func=AF.Exp,
                                         scale=w["A"][:, n:n + 1])
                    bc_t = bcw.tile([DI, 2 * HALF], BF16, name="bc_t",
                                    tag="bc_t")
                    nc.sync.dma_start(out=bc_t, in_=bass.AP(
                        tensor=bc_dram.tensor,
                        offset=bc_dram.offset + n * L + hs,
                        ap=[[0, DI], [DS * L, 2], [1, HALF]]))
                    dbu = nwork.tile([DI, HALF], BF16, name="dbu", tag="dbu")
                    veng.tensor_tensor(out=dbu, in0=dtu_t[:, qsl],
                                       in1=bc_t[:, 0:HALF], op=OP.mult)
                    h_t = nwork.tile([DI, HALF], BF16, name="h_t", tag="h_t")
                    init = 0.0 if q == 0 else carry_t[:, n:n + 1]
                    veng.tensor_tensor_scan(
                        out=h_t, data0=en, data1=dbu,
                        initial=init, op0=OP.mult, op1=OP.add)
                    if q == 0:
                        veng.tensor_copy(out=carry_t[:, n:n + 1],
                                         in_=h_t[:, HALF - 1:HALF])
                    hc = nwork.tile([DI, HALF], BF16, name="hc", tag="hc")
                    veng.tensor_tensor(out=hc, in0=h_t,
                                       in1=bc_t[:, HALF:2 * HALF],
                                       op=OP.mult)
                    for k in range(HALF // TC):
                        nc.tensor.matmul(yps[k], lhsT=ident,
                                         rhs=hc[:, k * TC:(k + 1) * TC],
                                         start=(n == 0), stop=(n == DS - 1))
                for k in range(HALF // TC):
                    cs = slice(hs + k * TC, hs + (k + 1) * TC)
                    ks = slice(k * TC, (k + 1) * TC)
                    tmp = small.tile([DI, TC], BF16, name="ytmp", tag="ytmp")
                    nc.vector.scalar_tensor_tensor(
                        out=tmp, in0=u_t[:, cs], scalar=w["D"][:, :],
                        in1=yps[k], op0=OP.mult, op1=OP.add)
                    nc.vector.tensor_tensor(out=yo_h[:, ks], in0=tmp,
                                            in1=zs_t[:, cs], op=OP.mult)

        def post_half(ui, h2):
            s, m = units[ui]
            w = blk[m]
            f2x = feat2x[s]
            yo_h = UST[ui][f"yo{h2}"]
            with nc.named_scope(f"s{s}m{m}_post{h2}"):
                fchsq = postw.tile([2 * DM, HALF], BF16, name="fchsq",
                                   tag="fchsq")
                # mean rows 0:4, mean-square rows 32:36 (PSUM reads must
                # start at a 32-partition boundary)
                ps8 = psB.tile([36, TC], F32, name="ps8", tag="ps8")
                for cc in range(4):
                    ls = slice(cc * TC, (cc + 1) * TC)
                    fps = psA.tile([DM, TC], F32, name="fps", tag="mm")
                    nc.tensor.matmul(fps, lhsT=w["opwT"], rhs=yo_h[:, ls],
                                     start=True, stop=True)
                    # square on GpSimd (from the SBUF copy; GpSimd cannot
                    # read PSUM): halves the post load on the in-order ACT
                    nc.scalar.activation(out=fchsq[0:DM, ls], in_=fps,
                                         func=AF.Copy)
                    nc.gpsimd.tensor_tensor(out=fchsq[DM:2 * DM, ls],
                                            in0=fchsq[0:DM, ls],
                                            in1=fchsq[0:DM, ls], op=OP.mult)
                    nc.tensor.matmul(ps8, lhsT=wsel8[:, cc * 36:(cc + 1) * 36],
                                     rhs=fchsq[:, ls],
                                     start=(cc == 0), stop=(cc == 3))
                # stats rows 0:4 = mu, rows 32:36 = rstd
                stats = postw.tile([36, TC], F32, name="stats", tag="st8")
                nc.scalar.activation(out=stats[0:4, :], in_=ps8[0:4, :],
                                     func=AF.Copy)
                mu2 = stp.tile([4, TC], F32, name="mu2", tag="mu2")
                nc.vector.tensor_tensor(out=mu2, in0=stats[0:4, :],
                                        in1=stats[0:4, :], op=OP.mult)
                var4 = stp.tile([4, TC], F32, name="var4", tag="var")
                nc.vector.tensor_tensor(out=var4, in0=ps8[32:36, :], in1=mu2,
                                        op=OP.subtract)
                # rstd = (var+eps)^-1/2 = exp(-0.5*ln(var+eps)); Ln and Exp
                # coexist in one ACT table (no Sqrt table switch)
                lnv = stp.tile([4, TC], F32, name="lnv", tag="lnv")
                nc.scalar.activation(out=lnv, in_=var4, func=AF.Ln,
                                     bias=eps8[0:4, :])
                nc.scalar.activation(out=stats[32:36, :], in_=lnv,
                                     func=AF.Exp, scale=-0.5)
                for cc in range(4):
                    c = h2 * 4 + cc
                    cs = slice(c * TC, (c + 1) * TC)
                    ls = slice(cc * TC, (cc + 1) * TC)
                    mrbc = psA.tile([2 * DM, TC], F32, name="mrbc", tag="mm")
                    nc.tensor.matmul(mrbc,
                                     lhsT=oh8[:, cc * 128:(cc + 1) * 128],
                                     rhs=stats, start=True, stop=True)
                    t1 = small.tile([DM, TC], BF16, name="t1", tag="t1")
                    nc.vector.tensor_tensor(out=t1, in0=fchsq[0:DM, ls],
                                            in1=mrbc[0:DM, :], op=OP.subtract)
                    t2 = small.tile([DM, TC], BF16, name="t2", tag="t2")
                    nc.vector.tensor_tensor(out=t2, in0=t1,
                                            in1=mrbc[DM:2 * DM, :], op=OP.mult)
                    if m == 0:
                        nc.scalar.activation(
                            out=f2x[0:DM, 3 + c * TC:3 + (c + 1) * TC],
                            in_=t2, func=AF.Identity,
                            scale=w["ln_g"][:, :], bias=w["ln_b"][:, :])
                        nc.scalar.activation(
                            out=f2x[DM:2 * DM, 2 + c * TC:2 + (c + 1) * TC],
                            in_=t2, func=AF.Identity,
                            scale=w["ln_g"][:, :], bias=w["ln_b"][:, :])
                    else:
                        # head input staged in a temp tile so the next
                        # sample's embed never waits on feat2x
                        hd = small.tile([DM, TC], BF16, name="hd", tag="hd")
                        nc.scalar.activation(
                            out=hd, in_=t2, func=AF.Identity,
                            scale=w["ln_g"][:, :], bias=w["ln_b"][:, :])
                        dps = psA.tile([C, TC], F32, name="dps", tag="mm")
                        nc.tensor.matmul(dps, lhsT=sb_headT, rhs=hd,
                                         start=True, stop=True)
                        nd = small.tile([C, TC], BF16, name="nd", tag="nd")
                        nc.scalar.activation(out=nd, in_=dps,
                                             func=AF.Identity,
                                             scale=-1.0, bias=sb_nhb[:, :])
                        zch2 = small.tile([C, TC], F32, name="zch2",
                                          tag="zch")
                        nc.scalar.dma_start(out=zch2, in_=zc[s][:, cs])
                        oc = small.tile([C, TC], F32, name="oc", tag="zch")
                        nc.vector.tensor_tensor(out=oc, in0=zch2, in1=nd,
                                                op=OP.add)
                        nc.gpsimd.dma_start(out=out[s][:, cs], in_=oc)

        # ---- software-pipelined emission ----
        # unit 0 startup split by half so the first scan starts sooner
        proj_begin(0)
        emit_embed(0, range(0, NCH // 2))
        proj_silu(0, range(0, NCH // 2))
        proj_xp(0, 0)
        for k in range(NU):
            scan_half(k, 0)
            if k == 0:
                emit_embed(0, range(NCH // 2, NCH))
                proj_silu(0, range(NCH // 2, NCH))
                proj_xp(0, 1)
                emit_embed(1)
            if k > 0:
                post_half(k - 1, 1)
            if k + 1 < NU:
                proj_full(k + 1)
            post_half(k, 0)
            scan_half(k, 1)
        post_half(NU - 1, 1)

    nc.finalize()
    return nc


def _prep_maps(inputs):
    import ml_dtypes
    bf = ml_dtypes.bfloat16
    f = np.float32
    z = np.asarray(inputs["z_damaged"], dtype=f).reshape(B, C, L)

    # oh8: per-cc [36,128] broadcast weights: out rows 0:64 get mu[cc],
    # rows 64:128 get rstd[cc]  (stats rows 0:4 = mu, 32:36 = rstd)
    oh8 = np.zeros((36, 4 * 128), f)
    for cc in range(4):
        oh8[cc, cc * 128:cc * 128 + 64] = 1.0
        oh8[32 + cc, cc * 128 + 64:cc * 128 + 128] = 1.0
    # wsel8: per-cc [128,36]: col cc sums rows 0:64 (x)/64, col 32+cc
    # sums rows 64:128 (x^2)/64 (msq lands at PSUM partition 32)
    wsel8 = np.zeros((128, 4 * 36), f)
    for cc in range(4):
        wsel8[0:64, cc * 36 + cc] = 1.0 / DM
        wsel8[64:128, cc * 36 + 32 + cc] = 1.0 / DM

    base = {
        "ident": np.eye(128, dtype=bf),
        "emb_wT": np.ascontiguousarray(np.asarray(inputs["emb_w"], f).T),
        "emb_b": np.asarray(inputs["emb_b"], f).reshape(DM, 1),
        "head_wT": np.ascontiguousarray(np.asarray(inputs["head_w"], f).T).astype(bf),
        "neg_head_b": (-np.asarray(inputs["head_b"], f)).reshape(C, 1),
        "oh8": oh8,
        "wsel8": wsel8.astype(bf),
    }
    for m in (1, 2):
        p = f"m{m}_"
        inw = np.asarray(inputs[p + "in_proj_w"], f)  # [2DI, DM]
        w_u = inw[:DI]  # [DI, DM]
        cw = np.asarray(inputs[p + "conv_w"], f).reshape(DI, DK)
        # lhsT rows (k,m) -> cols d: w[d,k]*W_u[d,m]
        base[p + "cwu0"] = np.ascontiguousarray(np.concatenate(
            [cw[:, 0][None, :] * w_u.T, cw[:, 1][None, :] * w_u.T], axis=0)).astype(bf)
        base[p + "cwu1"] = np.ascontiguousarray(np.concatenate(
            [cw[:, 2][None, :] * w_u.T, cw[:, 3][None, :] * w_u.T], axis=0)).astype(bf)
        base[p + "inw_zT"] = np.ascontiguousarray(inw[DI:].T).astype(bf)
        base[p + "conv_b"] = np.asarray(inputs[p + "conv_b"], f).reshape(DI, 1)
        xpw = np.asarray(inputs[p + "x_proj_w"], f)  # rows: dt(4), B(16), C(16)
        xpw = np.concatenate([xpw[DR:], xpw[:DR]], axis=0)  # -> B, C, dt
        base[p + "xpwT"] = np.ascontiguousarray(xpw.T).astype(bf)
        base[p + "dtpwT"] = np.ascontiguousarray(
            np.asarray(inputs[p + "dt_proj_w"], f).T).astype(bf)
        base[p + "dtp_b"] = np.asarray(inputs[p + "dt_proj_b"], f).reshape(DI, 1)
        base[p + "A"] = -np.exp(np.asarray(inputs[p + "A_log"], f))
        base[p + "D"] = np.asarray(inputs[p + "D"], f).reshape(DI, 1)
        base[p + "opwT"] = np.ascontiguousarray(
            np.asarray(inputs[p + "out_proj_w"], f).T).astype(bf)
        base[p + "ln_g"] = np.asarray(inputs[f"ln{m}_g"], f).reshape(DM, 1)
        base[p + "ln_b"] = np.asarray(inputs[f"ln{m}_b"], f).reshape(DM, 1)

    maps = []
    for k in range(NCORES):
        mkp = dict(base)
        mkp["zc"] = np.ascontiguousarray(z[k * BPC:(k + 1) * BPC])
        maps.append(mkp)
    return maps


def _run(inputs, trace=False):
    from concourse.bass_utils import run_bass_kernel_spmd
    if "nc" not in _CACHE:
        _CACHE["nc"] = _build_program()
    nc = _CACHE["nc"]
    maps = _prep_maps(inputs)
    res = run_bass_kernel_spmd(nc, maps, core_ids=list(range(NCORES)), trace=trace)
    outs = [r["out"] for r in res.results]
    full = np.concatenate(outs, axis=0).reshape(B, C, H, W)
    return full, res


def kernel(**inputs):
    full, _ = _run(inputs, trace=False)
    return full


# revision 20
# speedup vs baseline: 1.0247x; 1.0207x over previous
"""Trainium2 Bass kernel for nn_DriftRectifier (2-block Mamba over 64x64 images).

Sharding: data-parallel over batch B=16 -> 2 samples per core x 8 cores.
v4 architecture (sample-interleaved units + software-pipelined emission):
  - Unit order s0m0, s1m0, s0m1, s1m1: consecutive units are data-
    independent, so unit k+1's proj overlaps unit k's scan.
  - Emission order per k:  S0(k) | T1(k-1) | P(k+1) | T0(k) | S1(k)
    keeps the in-order ACT engine feeding scan exps back-to-back across
    unit boundaries (post/proj ACT work sits in scan-phase slack).
  - Vector (DVE): the 16-n selective scans (tensor_tensor_scan), dbu/hc
    multiplies, gate, LN small ops.
  - Scalar (ACT): exp/silu/softplus(exp+ln batched per half), stats.
    rstd = exp(-0.5*ln(var+eps)) so Ln/Exp stay in ONE table; no Sqrt.
  - GpSimd: post out_proj PSUM->SBUF copy + square (off the ACT critical
    path), output DMA.
  - PE: projections, per-n y accumulation via identity matmuls, LN mean
    and mean-square row-sums in ONE matmul (stacked [x; x^2] rhs), mu+rstd
    [1->64] broadcasts in ONE matmul.
  - DMA: B/C rows partition-broadcast from a DRAM staging tile, one
    descriptor per (n, half) via a 3-dim access pattern.
"""
import contextlib

import numpy as np

B, C, H, W = 16, 4, 64, 64
L = H * W  # 4096
DM, DI, DS, DK, DR = 64, 128, 16, 4, 4
NCORES = 8
BPC = B // NCORES  # samples per core
TC = 512           # psum / matmul chunk
NCH = L // TC      # 8 chunks
HALF = L // 2      # 2048, scan half-sequence
EPS = 1e-5

_CACHE = {}


def _build_program():
    import concourse.bacc as bacc
    import concourse.bass as bass
    from concourse import mybir
    from concourse.tile import TileContext

    F32 = mybir.dt.float32
    BF16 = mybir.dt.bfloat16
    AF = mybir.ActivationFunctionType
    OP = mybir.AluOpType

    nc = bacc.Bacc("TRN2")

    # ---- dram I/O ----
    zc = nc.dram_tensor("zc", [BPC, C, L], F32, kind="ExternalInput")
    out = nc.dram_tensor("out", [BPC, C, L], F32, kind="ExternalOutput")
    ident_in = nc.dram_tensor("ident", [128, 128], BF16, kind="ExternalInput")
    emb_wT = nc.dram_tensor("emb_wT", [C, DM], F32, kind="ExternalInput")
    emb_b = nc.dram_tensor("emb_b", [DM, 1], F32, kind="ExternalInput")
    head_wT = nc.dram_tensor("head_wT", [DM, C], BF16, kind="ExternalInput")
    neg_head_b = nc.dram_tensor("neg_head_b", [C, 1], F32, kind="ExternalInput")
    oh8_in = nc.dram_tensor("oh8", [36, 4 * 128], F32, kind="ExternalInput")
    wsel8_in = nc.dram_tensor("wsel8", [128, 4 * 36], BF16, kind="ExternalInput")
    blk_t = []
    for m in (1, 2):
        p = f"m{m}_"
        blk_t.append({
            "cwu0": nc.dram_tensor(p + "cwu0", [2 * DM, DI], BF16, kind="ExternalInput"),
            "cwu1": nc.dram_tensor(p + "cwu1", [2 * DM, DI], BF16, kind="ExternalInput"),
            "inw_zT": nc.dram_tensor(p + "inw_zT", [DM, DI], BF16, kind="ExternalInput"),
            "conv_b": nc.dram_tensor(p + "conv_b", [DI, 1], F32, kind="ExternalInput"),
            "xpwT": nc.dram_tensor(p + "xpwT", [DI, DR + 2 * DS], BF16, kind="ExternalInput"),
            "dtpwT": nc.dram_tensor(p + "dtpwT", [DR, DI], BF16, kind="ExternalInput"),
            "dtp_b": nc.dram_tensor(p + "dtp_b", [DI, 1], F32, kind="ExternalInput"),
            "A": nc.dram_tensor(p + "A", [DI, DS], F32, kind="ExternalInput"),
            "D": nc.dram_tensor(p + "D", [DI, 1], F32, kind="ExternalInput"),
            "opwT": nc.dram_tensor(p + "opwT", [DI, DM], BF16, kind="ExternalInput"),
            "ln_g": nc.dram_tensor(p + "ln_g", [DM, 1], F32, kind="ExternalInput"),
            "ln_b": nc.dram_tensor(p + "ln_b", [DM, 1], F32, kind="ExternalInput"),
        })

    with TileContext(nc) as tc, contextlib.ExitStack() as ctx:
        consts = ctx.enter_context(tc.tile_pool(name="consts", bufs=1))
        persist = ctx.enter_context(tc.tile_pool(name="persist", bufs=1))
        bcw = ctx.enter_context(tc.tile_pool(name="bcw", bufs=2))
        enw = ctx.enter_context(tc.tile_pool(name="enw", bufs=3))
        nwork = ctx.enter_context(tc.tile_pool(name="nwork", bufs=2))
        small = ctx.enter_context(tc.tile_pool(name="small", bufs=2))
        stp = ctx.enter_context(tc.tile_pool(name="stp", bufs=1))
        postw = ctx.enter_context(tc.tile_pool(name="postw", bufs=2))
        psA = ctx.enter_context(tc.tile_pool(name="psA", bufs=2, space="PSUM"))
        psB = ctx.enter_context(tc.tile_pool(name="psB", bufs=2, space="PSUM"))
        psY = ctx.enter_context(tc.tile_pool(name="psY", bufs=1, space="PSUM"))
        dstage = ctx.enter_context(tc.tile_pool(name="dstage", bufs=4, space="DRAM"))

        # ---- constants to SBUF ----
        ident = consts.tile([128, 128], BF16)
        nc.sync.dma_start(out=ident, in_=ident_in[:])
        sb_embT = consts.tile([C, DM], F32)
        nc.sync.dma_start(out=sb_embT, in_=emb_wT[:])
        sb_embb = consts.tile([DM, 1], F32)
        nc.sync.dma_start(out=sb_embb, in_=emb_b[:])
        sb_headT = consts.tile([DM, C], BF16)
        nc.sync.dma_start(out=sb_headT, in_=head_wT[:])
        sb_nhb = consts.tile([C, 1], F32)
        nc.sync.dma_start(out=sb_nhb, in_=neg_head_b[:])
        eps8 = consts.tile([NCH, 1], F32)
        nc.vector.memset(eps8, EPS)
        oh8 = consts.tile([36, 4 * 128], F32)
        nc.sync.dma_start(out=oh8, in_=oh8_in[:])
        wsel8 = consts.tile([128, 4 * 36], BF16)
        nc.sync.dma_start(out=wsel8, in_=wsel8_in[:])
        blk = []
        for m in range(2):
            d = {}
            for k, t in blk_t[m].items():
                d[k] = consts.tile(list(t.shape), t.dtype, name=f"c_m{m}_{k}")
                nc.sync.dma_start(out=d[k], in_=t[:])
            blk.append(d)

        # ---- persistent tiles ----
        # feat2x: PER-SAMPLE (embed / post-LN writes, next block's proj reads)
        feat2x = [persist.tile([2 * DM, L + 3], BF16, name=f"feat2x{i}")
                  for i in range(2)]
        # parity-duplicated so unit k+1's proj can overlap unit k's scan/post
        u_bf = [persist.tile([DI, L], BF16, name=f"u{i}") for i in range(2)]
        zs_bf = [persist.tile([DI, L], BF16, name=f"zs{i}") for i in range(2)]
        dtu_bf = [persist.tile([DI, L], BF16, name=f"dtu{i}") for i in range(2)]
        dt_f32 = [persist.tile([DI, L], BF16, name=f"dt{i}") for i in range(2)]
        carry = [persist.tile([DI, DS], F32, name=f"carry{i}") for i in range(2)]

        def emit_embed(si, crange=None):
            with nc.named_scope(f"s{si}_embed"):
                for c in (crange if crange is not None else range(NCH)):
                    cs = slice(c * TC, (c + 1) * TC)
                    zch = small.tile([C, TC], F32, name="zch", tag="zch")
                    nc.scalar.dma_start(out=zch, in_=zc[si][:, cs])
                    ps = psA.tile([DM, TC], F32, name="emb_ps", tag="mm")
                    nc.tensor.matmul(ps, lhsT=sb_embT, rhs=zch,
                                     start=True, stop=True)
                    nc.scalar.activation(
                        out=feat2x[si][0:DM, 3 + c * TC:3 + (c + 1) * TC],
                        in_=ps, func=AF.Identity, bias=sb_embb[:, :])
                    nc.scalar.activation(
                        out=feat2x[si][DM:2 * DM, 2 + c * TC:2 + (c + 1) * TC],
                        in_=ps, func=AF.Identity, bias=sb_embb[:, :])
                if 0 in (crange or range(NCH)):
                    nc.vector.memset(feat2x[si][0:DM, 0:3], 0.0)
                    nc.vector.memset(feat2x[si][DM:2 * DM, 0:2], 0.0)

        # n-values whose scan chain runs on GpSimd (second scan engine)
        POOL_NS = frozenset()
        # sample-interleaved unit order: consecutive units are independent
        units = [(0, 0), (1, 0), (0, 1), (1, 1)]
        NU = len(units)
        UST = [{} for _ in range(NU)]

        def proj_begin(ui):
            UST[ui]["bc"] = dstage.tile([2 * DS, L], BF16, name="bc_dram")

        def proj_silu(ui, crange):
            s, m = units[ui]
            w = blk[m]
            par = ui % 2
            u_t, zs_t = u_bf[par], zs_bf[par]
            f2x = feat2x[s]
            with nc.named_scope(f"s{s}m{m}_proj"):
                # silu pass (conv fused into in_proj via shifted feat2x)
                for c in crange:
                    cs = slice(c * TC, (c + 1) * TC)
                    ups = psA.tile([DI, TC], F32, name="ups", tag="mm")
                    nc.tensor.matmul(ups, lhsT=w["cwu0"],
                                     rhs=f2x[:, c * TC:c * TC + TC],
                                     start=True, stop=False)
                    nc.tensor.matmul(ups, lhsT=w["cwu1"],
                                     rhs=f2x[:, c * TC + 2:c * TC + 2 + TC],
                                     start=False, stop=True)
                    nc.scalar.activation(out=u_t[:, cs], in_=ups, func=AF.Silu,
                                         bias=w["conv_b"][:, :])
                    zps = psA.tile([DI, TC], F32, name="zps", tag="mm")
                    nc.tensor.matmul(zps, lhsT=w["inw_zT"],
                                     rhs=f2x[0:DM, 3 + c * TC:3 + (c + 1) * TC],
                                     start=True, stop=True)
                    nc.scalar.activation(out=zs_t[:, cs], in_=zps, func=AF.Silu)

        def proj_xp(ui, half):
            s, m = units[ui]
            w = blk[m]
            par = ui % 2
            u_t, dt_t = u_bf[par], dt_f32[par]
            dtu_t = dtu_bf[par]
            bc_dram = UST[ui]["bc"]
            # x_proj / dt pass: exps batched per half, then one ln
            with nc.named_scope(f"s{s}m{m}_proj"):
                if True:
                    spe = enw.tile([DI, HALF], F32, name="spe", tag="en")
                    for cc in range(NCH // 2):
                        c = half * (NCH // 2) + cc
                        cs = slice(c * TC, (c + 1) * TC)
                        xps = psA.tile([DR + 2 * DS, TC], F32, name="xps", tag="mm")
                        nc.tensor.matmul(xps, lhsT=w["xpwT"], rhs=u_t[:, cs],
                                         start=True, stop=True)
                        # x_proj rows host-permuted to [B(16), C(16), dt(4)]
                        bcc = small.tile([2 * DS, TC], BF16, name="bcc", tag="bcc")
                        nc.scalar.activation(out=bcc, in_=xps[0:2 * DS, :],
                                             func=AF.Copy)
                        nc.sync.dma_start(out=bc_dram[:, cs], in_=bcc)
                        dtr = small.tile([DR, TC], BF16, name="dtr", tag="dtr")
                        nc.scalar.activation(out=dtr,
                                             in_=xps[2 * DS:2 * DS + DR, :],
                                             func=AF.Copy)
                        dtps = psA.tile([DI, TC], F32, name="dtps", tag="mm")
                        nc.tensor.matmul(dtps, lhsT=w["dtpwT"], rhs=dtr,
                                         start=True, stop=True)
                        # softplus(x) = ln(1 + exp(x))
                        nc.scalar.activation(out=spe[:, cc * TC:(cc + 1) * TC],
                                             in_=dtps, func=AF.Exp,
                                             bias=w["dtp_b"][:, :])
                    hsl = slice(half * HALF, (half + 1) * HALF)
                    nc.scalar.activation(out=dt_t[:, hsl],
                                         in_=spe, func=AF.Ln, bias=1.0)
                    for j in range(2):
                        qj = slice(half * HALF + j * (HALF // 2),
                                   half * HALF + (j + 1) * (HALF // 2))
                        nc.vector.tensor_tensor(out=dtu_t[:, qj], in0=dt_t[:, qj],
                                                in1=u_t[:, qj], op=OP.mult)

        def proj_full(ui):
            proj_begin(ui)
            proj_silu(ui, range(NCH))
            proj_xp(ui, 0)
            proj_xp(ui, 1)

        def scan_half(ui, q):
            s, m = units[ui]
            w = blk[m]
            par = ui % 2
            u_t, zs_t, dtu_t, dt_t = u_bf[par], zs_bf[par], dtu_bf[par], dt_f32[par]
            carry_t = carry[par]
            bc_dram = UST[ui]["bc"]
            with nc.named_scope(f"s{s}m{m}_scan{q}"):
                hs = q * HALF
                qsl = slice(hs, hs + HALF)
                yo_h = postw.tile([DI, HALF], BF16, name="yo_h", tag="yo")
                UST[ui][f"yo{q}"] = yo_h
                yps = [psY.tile([DI, TC], F32, name=f"yps{k}", tag=f"yps{k}")
                       for k in range(HALF // TC)]
                for n in range(DS):
                    veng = nc.gpsimd if n in POOL_NS else nc.vector
                    en = enw.tile([DI, HALF], F32, name="en", tag="en")
                    nc.scalar.activation(out=en, in_=dt_t[:, qsl],
                                         func=AF.Exp,
                                         scale=w["A"][:, n:n + 1])
                    bc_t = bcw.tile([DI, 2 * HALF], BF16, name="bc_t",
                                    tag="bc_t")
                    nc.sync.dma_start(out=bc_t, in_=bass.AP(
                        tensor=bc_dram.tensor,
                        offset=bc_dram.offset + n * L + hs,
                        ap=[[0, DI], [DS * L, 2], [1, HALF]]))
                    dbu = nwork.tile([DI, HALF], BF16, name="dbu", tag="dbu")
                    veng.tensor_tensor(out=dbu, in0=dtu_t[:, qsl],
                                       in1=bc_t[:, 0:HALF], op=OP.mult)
                    h_t = nwork.tile([DI, HALF], BF16, name="h_t", tag="h_t")
                    init = 0.0 if q == 0 else carry_t[:, n:n + 1]
                    veng.tensor_tensor_scan(
                        out=h_t, data0=en, data1=dbu,
                        initial=init, op0=OP.mult, op1=OP.add)
                    if q == 0:
                        veng.tensor_copy(out=carry_t[:, n:n + 1],
                                         in_=h_t[:, HALF - 1:HALF])
                    hc = nwork.tile([DI, HALF], BF16, name="hc", tag="hc")
                    veng.tensor_tensor(out=hc, in0=h_t,
                                       in1=bc_t[:, HALF:2 * HALF],
                                       op=OP.mult)
                    for k in range(HALF // TC):
                        nc.tensor.matmul(yps[k], lhsT=ident,
                                         rhs=hc[:, k * TC:(k + 1) * TC],
                                         start=(n == 0), stop=(n == DS - 1))
                for k in range(HALF // TC):
                    cs = slice(hs + k * TC, hs + (k + 1) * TC)
                    ks = slice(k * TC, (k + 1) * TC)
                    tmp = small.tile([DI, TC], BF16, name="ytmp", tag="ytmp")
                    nc.vector.scalar_tensor_tensor(
                        out=tmp, in0=u_t[:, cs], scalar=w["D"][:, :],
                        in1=yps[k], op0=OP.mult, op1=OP.add)
                    nc.vector.tensor_tensor(out=yo_h[:, ks], in0=tmp,
                                            in1=zs_t[:, cs], op=OP.mult)

        def post_half(ui, h2):
            s, m = units[ui]
            w = blk[m]
            f2x = feat2x[s]
            yo_h = UST[ui][f"yo{h2}"]
            with nc.named_scope(f"s{s}m{m}_post{h2}"):
                fchsq = postw.tile([2 * DM, HALF], BF16, name="fchsq",
                                   tag="fchsq")
                # mean rows 0:4, mean-square rows 32:36 (PSUM reads must
                # start at a 32-partition boundary)
                ps8 = psB.tile([36, TC], F32, name="ps8", tag="ps8")
                for cc in range(4):
                    ls = slice(cc * TC, (cc + 1) * TC)
                    fps = psA.tile([DM, TC], F32, name="fps", tag="mm")
                    nc.tensor.matmul(fps, lhsT=w["opwT"], rhs=yo_h[:, ls],
                                     start=True, stop=True)
                    # square on GpSimd (from the SBUF copy; GpSimd cannot
                    # read PSUM): halves the post load on the in-order ACT
                    nc.scalar.activation(out=fchsq[0:DM, ls], in_=fps,
                                         func=AF.Copy)
                    nc.gpsimd.tensor_tensor(out=fchsq[DM:2 * DM, ls],
                                            in0=fchsq[0:DM, ls],
                                            in1=fchsq[0:DM, ls], op=OP.mult)
                    nc.tensor.matmul(ps8, lhsT=wsel8[:, cc * 36:(cc + 1) * 36],
                                     rhs=fchsq[:, ls],
                                     start=(cc == 0), stop=(cc == 3))
                # stats rows 0:4 = mu, rows 32:36 = rstd
                stats = postw.tile([36, TC], F32, name="stats", tag="st8")
                nc.scalar.activation(out=stats[0:4, :], in_=ps8[0:4, :],
                                     func=AF.Copy)
                mu2 = stp.tile([4, TC], F32, name="mu2", tag="mu2")
                nc.vector.tensor_tensor(out=mu2, in0=stats[0:4, :],
                                        in1=stats[0:4, :], op=OP.mult)
                var4 = stp.tile([4, TC], F32, name="var4", tag="var")
                nc.vector.tensor_tensor(out=var4, in0=ps8[32:36, :], in1=mu2,
                                        op=OP.subtract)
                # rstd = (var+eps)^-1/2 = exp(-0.5*ln(var+eps)); Ln and Exp
                # coexist in one ACT table (no Sqrt table switch)
                lnv = stp.tile([4, TC], F32, name="lnv", tag="lnv")
                nc.scalar.activation(out=lnv, in_=var4, func=AF.Ln,
                                     bias=eps8[0:4, :])
                nc.scalar.activation(out=stats[32:36, :], in_=lnv,
                                     func=AF.Exp, scale=-0.5)
                for cc in range(4):
                    c = h2 * 4 + cc
                    cs = slice(c * TC, (c + 1) * TC)
                    ls = slice(cc * TC, (cc + 1) * TC)
                    mrbc = psA.tile([2 * DM, TC], F32, name="mrbc", tag="mm")
                    nc.tensor.matmul(mrbc,
                                     lhsT=oh8[:, cc * 128:(cc + 1) * 128],
                                     rhs=stats, start=True, stop=True)
                    t1 = small.tile([DM, TC], BF16, name="t1", tag="t1")
                    nc.vector.tensor_tensor(out=t1, in0=fchsq[0:DM, ls],
                                            in1=mrbc[0:DM, :], op=OP.subtract)
                    t2 = small.tile([DM, TC], BF16, name="t2", tag="t2")
                    nc.vector.tensor_tensor(out=t2, in0=t1,
                                            in1=mrbc[DM:2 * DM, :], op=OP.mult)
                    if m == 0:
                        nc.scalar.activation(
                            out=f2x[0:DM, 3 + c * TC:3 + (c + 1) * TC],
                            in_=t2, func=AF.Identity,
                            scale=w["ln_g"][:, :], bias=w["ln_b"][:, :])
                        nc.scalar.activation(
                            out=f2x[DM:2 * DM, 2 + c * TC:2 + (c + 1) * TC],
                            in_=t2, func=AF.Identity,
                            scale=w["ln_g"][:, :], bias=w["ln_b"][:, :])
                    else:
                        # head input staged in a temp tile so the next
                        # sample's embed never waits on feat2x
                        hd = small.tile([DM, TC], BF16, name="hd", tag="hd")
                        nc.scalar.activation(
                            out=hd, in_=t2, func=AF.Identity,
                            scale=w["ln_g"][:, :], bias=w["ln_b"][:, :])
                        dps = psA.tile([C, TC], F32, name="dps", tag="mm")
                        nc.tensor.matmul(dps, lhsT=sb_headT, rhs=hd,
                                         start=True, stop=True)
                        nd = small.tile([C, TC], BF16, name="nd", tag="nd")
                        nc.scalar.activation(out=nd, in_=dps,
                                             func=AF.Identity,
                                             scale=-1.0, bias=sb_nhb[:, :])
                        zch2 = small.tile([C, TC], F32, name="zch2",
                                          tag="zch")
                        nc.scalar.dma_start(out=zch2, in_=zc[s][:, cs])
                        oc = small.tile([C, TC], F32, name="oc", tag="zch")
                        nc.vector.tensor_tensor(out=oc, in0=zch2, in1=nd,
                                                op=OP.add)
                        nc.gpsimd.dma_start(out=out[s][:, cs], in_=oc)

        # ---- software-pipelined emission ----
        # unit 0 startup split by half so the first scan starts sooner
        proj_begin(0)
        emit_embed(0, range(0, NCH // 2))
        proj_silu(0, range(0, NCH // 2))
        proj_xp(0, 0)
        for k in range(NU):
            scan_half(k, 0)
            if k == 0:
                emit_embed(0, range(NCH // 2, NCH))
                proj_silu(0, range(NCH // 2, NCH))
                proj_xp(0, 1)
                emit_embed(1)
            if k > 0:
                post_half(k - 1, 1)
            if k + 1 < NU:
                proj_full(k + 1)
            post_half(k, 0)
            scan_half(k, 1)
        post_half(NU - 1, 1)

    nc.finalize()
    return nc


def _prep_maps(inputs):
    import ml_dtypes
    bf = ml_dtypes.bfloat16
    f = np.float32
    z = np.asarray(inputs["z_damaged"], dtype=f).reshape(B, C, L)

    # oh8: per-cc [36,128] broadcast weights: out rows 0:64 get mu[cc],
    # rows 64:128 get rstd[cc]  (stats rows 0:4 = mu, 32:36 = rstd)
    oh8 = np.zeros((36, 4 * 128), f)
    for cc in range(4):
        oh8[cc, cc * 128:cc * 128 + 64] = 1.0
        oh8[32 + cc, cc * 128 + 64:cc * 128 + 128] = 1.0
    # wsel8: per-cc [128,36]: col cc sums rows 0:64 (x)/64, col 32+cc
    # sums rows 64:128 (x^2)/64 (msq lands at PSUM partition 32)
    wsel8 = np.zeros((128, 4 * 36), f)
    for cc in range(4):
        wsel8[0:64, cc * 36 + cc] = 1.0 / DM
        wsel8[64:128, cc * 36 + 32 + cc] = 1.0 / DM

    base = {
        "ident": np.eye(128, dtype=bf),
        "emb_wT": np.ascontiguousarray(np.asarray(inputs["emb_w"], f).T),
        "emb_b": np.asarray(inputs["emb_b"], f).reshape(DM, 1),
        "head_wT": np.ascontiguousarray(np.asarray(inputs["head_w"], f).T).astype(bf),
        "neg_head_b": (-np.asarray(inputs["head_b"], f)).reshape(C, 1),
        "oh8": oh8,
        "wsel8": wsel8.astype(bf),
    }
    for m in (1, 2):
        p = f"m{m}_"
        inw = np.asarray(inputs[p + "in_proj_w"], f)  # [2DI, DM]
        w_u = inw[:DI]  # [DI, DM]
        cw = np.asarray(inputs[p + "conv_w"], f).reshape(DI, DK)
        # lhsT rows (k,m) -> cols d: w[d,k]*W_u[d,m]
        base[p + "cwu0"] = np.ascontiguousarray(np.concatenate(
            [cw[:, 0][None, :] * w_u.T, cw[:, 1][None, :] * w_u.T], axis=0)).astype(bf)
        base[p + "cwu1"] = np.ascontiguousarray(np.concatenate(
            [cw[:, 2][None, :] * w_u.T, cw[:, 3][None, :] * w_u.T], axis=0)).astype(bf)
        base[p + "inw_zT"] = np.ascontiguousarray(inw[DI:].T).astype(bf)
        base[p + "conv_b"] = np.asarray(inputs[p + "conv_b"], f).reshape(DI, 1)
        xpw = np.asarray(inputs[p + "x_proj_w"], f)  # rows: dt(4), B(16), C(16)
        xpw = np.concatenate([xpw[DR:], xpw[:DR]], axis=0)  # -> B, C, dt
        base[p + "xpwT"] = np.ascontiguousarray(xpw.T).astype(bf)
        base[p + "dtpwT"] = np.ascontiguousarray(
            np.asarray(inputs[p + "dt_proj_w"], f).T).astype(bf)
        base[p + "dtp_b"] = np.asarray(inputs[p + "dt_proj_b"], f).reshape(DI, 1)
        base[p + "A"] = -np.exp(np.asarray(inputs[p + "A_log"], f))
        base[p + "D"] = np.asarray(inputs[p + "D"], f).reshape(DI, 1)
        base[p + "opwT"] = np.ascontiguousarray(
            np.asarray(inputs[p + "out_proj_w"], f).T).astype(bf)
        base[p + "ln_g"] = np.asarray(inputs[f"ln{m}_g"], f).reshape(DM, 1)
        base[p + "ln_b"] = np.asarray(inputs[f"ln{m}_b"], f).reshape(DM, 1)

    maps = []
    for k in range(NCORES):
        mkp = dict(base)
        mkp["zc"] = np.ascontiguousarray(z[k * BPC:(k + 1) * BPC])
        maps.append(mkp)
    return maps


def _run(inputs, trace=False):
    from concourse.bass_utils import run_bass_kernel_spmd
    if "nc" not in _CACHE:
        _CACHE["nc"] = _build_program()
    nc = _CACHE["nc"]
    maps = _prep_maps(inputs)
    res = run_bass_kernel_spmd(nc, maps, core_ids=list(range(NCORES)), trace=trace)
    outs = [r["out"] for r in res.results]
    full = np.concatenate(outs, axis=0).reshape(B, C, H, W)
    return full, res


def kernel(**inputs):
    full, _ = _run(inputs, trace=False)
    return full


# revision 21
# speedup vs baseline: 1.0284x; 1.0036x over previous
"""Trainium2 Bass kernel for nn_DriftRectifier (2-block Mamba over 64x64 images).

Sharding: data-parallel over batch B=16 -> 2 samples per core x 8 cores.
v4 architecture (sample-interleaved units + software-pipelined emission):
  - Unit order s0m0, s1m0, s0m1, s1m1: consecutive units are data-
    independent, so unit k+1's proj overlaps unit k's scan.
  - Emission order per k:  S0(k) | T1(k-1) | P(k+1) | T0(k) | S1(k)
    keeps the in-order ACT engine feeding scan exps back-to-back across
    unit boundaries (post/proj ACT work sits in scan-phase slack).
  - Vector (DVE): the 16-n selective scans (tensor_tensor_scan), dbu/hc
    multiplies, gate, LN small ops.
  - Scalar (ACT): exp/silu/softplus(exp+ln batched per half), stats.
    rstd = exp(-0.5*ln(var+eps)) so Ln/Exp stay in ONE table; no Sqrt.
  - GpSimd: post out_proj PSUM->SBUF copy + square (off the ACT critical
    path), output DMA.
  - PE: projections, per-n y accumulation via identity matmuls, LN mean
    and mean-square row-sums in ONE matmul (stacked [x; x^2] rhs), mu+rstd
    [1->64] broadcasts in ONE matmul.
  - DMA: B/C rows partition-broadcast from a DRAM staging tile, one
    descriptor per (n, half) via a 3-dim access pattern.
"""
import contextlib

import numpy as np

B, C, H, W = 16, 4, 64, 64
L = H * W  # 4096
DM, DI, DS, DK, DR = 64, 128, 16, 4, 4
NCORES = 8
BPC = B // NCORES  # samples per core
TC = 512           # psum / matmul chunk
NCH = L // TC      # 8 chunks
HALF = L // 2      # 2048, scan half-sequence
EPS = 1e-5

_CACHE = {}


def _build_program():
    import concourse.bacc as bacc
    import concourse.bass as bass
    from concourse import mybir
    from concourse.tile import TileContext

    F32 = mybir.dt.float32
    BF16 = mybir.dt.bfloat16
    AF = mybir.ActivationFunctionType
    OP = mybir.AluOpType

    nc = bacc.Bacc("TRN2")

    # ---- dram I/O ----
    zc = nc.dram_tensor("zc", [BPC, C, L], F32, kind="ExternalInput")
    out = nc.dram_tensor("out", [BPC, C, L], F32, kind="ExternalOutput")
    ident_in = nc.dram_tensor("ident", [128, 128], BF16, kind="ExternalInput")
    emb_wT = nc.dram_tensor("emb_wT", [C, DM], F32, kind="ExternalInput")
    emb_b = nc.dram_tensor("emb_b", [DM, 1], F32, kind="ExternalInput")
    head_wT = nc.dram_tensor("head_wT", [DM, C], BF16, kind="ExternalInput")
    neg_head_b = nc.dram_tensor("neg_head_b", [C, 1], F32, kind="ExternalInput")
    oh8_in = nc.dram_tensor("oh8", [36, 4 * 128], F32, kind="ExternalInput")
    wsel8_in = nc.dram_tensor("wsel8", [128, 4 * 36], BF16, kind="ExternalInput")
    blk_t = []
    for m in (1, 2):
        p = f"m{m}_"
        blk_t.append({
            "cwu0": nc.dram_tensor(p + "cwu0", [2 * DM, DI], BF16, kind="ExternalInput"),
            "cwu1": nc.dram_tensor(p + "cwu1", [2 * DM, DI], BF16, kind="ExternalInput"),
            "inw_zT": nc.dram_tensor(p + "inw_zT", [DM, DI], BF16, kind="ExternalInput"),
            "conv_b": nc.dram_tensor(p + "conv_b", [DI, 1], F32, kind="ExternalInput"),
            "xpwT": nc.dram_tensor(p + "xpwT", [DI, DR + 2 * DS], BF16, kind="ExternalInput"),
            "dtpwT": nc.dram_tensor(p + "dtpwT", [DR, DI], BF16, kind="ExternalInput"),
            "dtp_b": nc.dram_tensor(p + "dtp_b", [DI, 1], F32, kind="ExternalInput"),
            "A": nc.dram_tensor(p + "A", [DI, DS], F32, kind="ExternalInput"),
            "D": nc.dram_tensor(p + "D", [DI, 1], F32, kind="ExternalInput"),
            "opwT": nc.dram_tensor(p + "opwT", [DI, DM], BF16, kind="ExternalInput"),
            "ln_g": nc.dram_tensor(p + "ln_g", [DM, 1], F32, kind="ExternalInput"),
            "ln_b": nc.dram_tensor(p + "ln_b", [DM, 1], F32, kind="ExternalInput"),
        })

    with TileContext(nc) as tc, contextlib.ExitStack() as ctx:
        consts = ctx.enter_context(tc.tile_pool(name="consts", bufs=1))
        persist = ctx.enter_context(tc.tile_pool(name="persist", bufs=1))
        bcw = ctx.enter_context(tc.tile_pool(name="bcw", bufs=2))
        enw = ctx.enter_context(tc.tile_pool(name="enw", bufs=3))
        nwork = ctx.enter_context(tc.tile_pool(name="nwork", bufs=2))
        small = ctx.enter_context(tc.tile_pool(name="small", bufs=2))
        stp = ctx.enter_context(tc.tile_pool(name="stp", bufs=1))
        postw = ctx.enter_context(tc.tile_pool(name="postw", bufs=2))
        psA = ctx.enter_context(tc.tile_pool(name="psA", bufs=2, space="PSUM"))
        psB = ctx.enter_context(tc.tile_pool(name="psB", bufs=2, space="PSUM"))
        psY = ctx.enter_context(tc.tile_pool(name="psY", bufs=1, space="PSUM"))
        dstage = ctx.enter_context(tc.tile_pool(name="dstage", bufs=4, space="DRAM"))

        # ---- constants to SBUF ----
        ident = consts.tile([128, 128], BF16)
        nc.sync.dma_start(out=ident, in_=ident_in[:])
        sb_embT = consts.tile([C, DM], F32)
        nc.sync.dma_start(out=sb_embT, in_=emb_wT[:])
        sb_embb = consts.tile([DM, 1], F32)
        nc.sync.dma_start(out=sb_embb, in_=emb_b[:])
        sb_headT = consts.tile([DM, C], BF16)
        nc.sync.dma_start(out=sb_headT, in_=head_wT[:])
        sb_nhb = consts.tile([C, 1], F32)
        nc.sync.dma_start(out=sb_nhb, in_=neg_head_b[:])
        eps8 = consts.tile([NCH, 1], F32)
        nc.vector.memset(eps8, EPS)
        oh8 = consts.tile([36, 4 * 128], F32)
        nc.sync.dma_start(out=oh8, in_=oh8_in[:])
        wsel8 = consts.tile([128, 4 * 36], BF16)
        nc.sync.dma_start(out=wsel8, in_=wsel8_in[:])
        blk = []
        for m in range(2):
            d = {}
            for k, t in blk_t[m].items():
                d[k] = consts.tile(list(t.shape), t.dtype, name=f"c_m{m}_{k}")
                nc.sync.dma_start(out=d[k], in_=t[:])
            blk.append(d)

        # ---- persistent tiles ----
        # feat2x: PER-SAMPLE (embed / post-LN writes, next block's proj reads)
        feat2x = [persist.tile([2 * DM, L + 3], BF16, name=f"feat2x{i}")
                  for i in range(2)]
        # parity-duplicated so unit k+1's proj can overlap unit k's scan/post
        u_bf = [persist.tile([DI, L], BF16, name=f"u{i}") for i in range(2)]
        zs_bf = [persist.tile([DI, L], BF16, name=f"zs{i}") for i in range(2)]
        dtu_bf = [persist.tile([DI, L], BF16, name=f"dtu{i}") for i in range(2)]
        dt_f32 = [persist.tile([DI, L], BF16, name=f"dt{i}") for i in range(2)]
        carry = [persist.tile([DI, DS], F32, name=f"carry{i}") for i in range(2)]

        def emit_embed(si, crange=None):
            with nc.named_scope(f"s{si}_embed"):
                for c in (crange if crange is not None else range(NCH)):
                    cs = slice(c * TC, (c + 1) * TC)
                    zch = small.tile([C, TC], F32, name="zch", tag="zch")
                    nc.scalar.dma_start(out=zch, in_=zc[si][:, cs])
                    ps = psA.tile([DM, TC], F32, name="emb_ps", tag="mm")
                    nc.tensor.matmul(ps, lhsT=sb_embT, rhs=zch,
                                     start=True, stop=True)
                    nc.scalar.activation(
                        out=feat2x[si][0:DM, 3 + c * TC:3 + (c + 1) * TC],
                        in_=ps, func=AF.Identity, bias=sb_embb[:, :])
                    nc.scalar.activation(
                        out=feat2x[si][DM:2 * DM, 2 + c * TC:2 + (c + 1) * TC],
                        in_=ps, func=AF.Identity, bias=sb_embb[:, :])
                if 0 in (crange or range(NCH)):
                    nc.vector.memset(feat2x[si][0:DM, 0:3], 0.0)
                    nc.vector.memset(feat2x[si][DM:2 * DM, 0:2], 0.0)

        # n-values whose scan chain runs on GpSimd (second scan engine)
        POOL_NS = frozenset()
        # sample-interleaved unit order: consecutive units are independent
        units = [(0, 0), (1, 0), (0, 1), (1, 1)]
        NU = len(units)
        UST = [{} for _ in range(NU)]

        def proj_begin(ui):
            UST[ui]["bc"] = dstage.tile([2 * DS, L], BF16, name="bc_dram")

        def proj_silu(ui, crange):
            s, m = units[ui]
            w = blk[m]
            par = ui % 2
            u_t, zs_t = u_bf[par], zs_bf[par]
            f2x = feat2x[s]
            with nc.named_scope(f"s{s}m{m}_proj"):
                # silu pass (conv fused into in_proj via shifted feat2x)
                for c in crange:
                    cs = slice(c * TC, (c + 1) * TC)
                    ups = psA.tile([DI, TC], F32, name="ups", tag="mm")
                    nc.tensor.matmul(ups, lhsT=w["cwu0"],
                                     rhs=f2x[:, c * TC:c * TC + TC],
                                     start=True, stop=False)
                    nc.tensor.matmul(ups, lhsT=w["cwu1"],
                                     rhs=f2x[:, c * TC + 2:c * TC + 2 + TC],
                                     start=False, stop=True)
                    nc.scalar.activation(out=u_t[:, cs], in_=ups, func=AF.Silu,
                                         bias=w["conv_b"][:, :])
                    zps = psA.tile([DI, TC], F32, name="zps", tag="mm")
                    nc.tensor.matmul(zps, lhsT=w["inw_zT"],
                                     rhs=f2x[0:DM, 3 + c * TC:3 + (c + 1) * TC],
                                     start=True, stop=True)
                    nc.scalar.activation(out=zs_t[:, cs], in_=zps, func=AF.Silu)

        def proj_xp(ui, half):
            s, m = units[ui]
            w = blk[m]
            par = ui % 2
            u_t, dt_t = u_bf[par], dt_f32[par]
            dtu_t = dtu_bf[par]
            bc_dram = UST[ui]["bc"]
            # x_proj / dt pass: exps batched per half, then one ln
            with nc.named_scope(f"s{s}m{m}_proj"):
                if True:
                    spe = enw.tile([DI, HALF], F32, name="spe", tag="en")
                    for cc in range(NCH // 2):
                        c = half * (NCH // 2) + cc
                        cs = slice(c * TC, (c + 1) * TC)
                        xps = psA.tile([DR + 2 * DS, TC], F32, name="xps", tag="mm")
                        nc.tensor.matmul(xps, lhsT=w["xpwT"], rhs=u_t[:, cs],
                                         start=True, stop=True)
                        # x_proj rows host-permuted to [B(16), C(16), dt(4)]
                        bcc = small.tile([2 * DS, TC], BF16, name="bcc", tag="bcc")
                        nc.scalar.activation(out=bcc, in_=xps[0:2 * DS, :],
                                             func=AF.Copy)
                        nc.sync.dma_start(out=bc_dram[:, cs], in_=bcc)
                        dtr = small.tile([DR, TC], BF16, name="dtr", tag="dtr")
                        nc.scalar.activation(out=dtr,
                                             in_=xps[2 * DS:2 * DS + DR, :],
                                             func=AF.Copy)
                        dtps = psA.tile([DI, TC], F32, name="dtps", tag="mm")
                        nc.tensor.matmul(dtps, lhsT=w["dtpwT"], rhs=dtr,
                                         start=True, stop=True)
                        # softplus(x) = ln(1 + exp(x))
                        nc.scalar.activation(out=spe[:, cc * TC:(cc + 1) * TC],
                                             in_=dtps, func=AF.Exp,
                                             bias=w["dtp_b"][:, :])
                    hsl = slice(half * HALF, (half + 1) * HALF)
                    nc.scalar.activation(out=dt_t[:, hsl],
                                         in_=spe, func=AF.Ln, bias=1.0)
                    for j in range(2):
                        qj = slice(half * HALF + j * (HALF // 2),
                                   half * HALF + (j + 1) * (HALF // 2))
                        nc.vector.tensor_tensor(out=dtu_t[:, qj], in0=dt_t[:, qj],
                                                in1=u_t[:, qj], op=OP.mult)

        def proj_full(ui):
            proj_begin(ui)
            proj_silu(ui, range(NCH))
            proj_xp(ui, 0)
            proj_xp(ui, 1)

        def scan_half(ui, q):
            s, m = units[ui]
            w = blk[m]
            par = ui % 2
            u_t, zs_t, dtu_t, dt_t = u_bf[par], zs_bf[par], dtu_bf[par], dt_f32[par]
            carry_t = carry[par]
            bc_dram = UST[ui]["bc"]
            with nc.named_scope(f"s{s}m{m}_scan{q}"):
                hs = q * HALF
                qsl = slice(hs, hs + HALF)
                yo_h = postw.tile([DI, HALF], BF16, name="yo_h", tag="yo")
                UST[ui][f"yo{q}"] = yo_h
                yps = [psY.tile([DI, TC], F32, name=f"yps{k}", tag=f"yps{k}")
                       for k in range(HALF // TC)]
                for n in range(DS):
                    veng = nc.gpsimd if n in POOL_NS else nc.vector
                    en = enw.tile([DI, HALF], F32, name="en", tag="en")
                    nc.scalar.activation(out=en, in_=dt_t[:, qsl],
                                         func=AF.Exp,
                                         scale=w["A"][:, n:n + 1])
                    bc_t = bcw.tile([DI, 2 * HALF], BF16, name="bc_t",
                                    tag="bc_t")
                    nc.sync.dma_start(out=bc_t, in_=bass.AP(
                        tensor=bc_dram.tensor,
                        offset=bc_dram.offset + n * L + hs,
                        ap=[[0, DI], [DS * L, 2], [1, HALF]]))
                    dbu = nwork.tile([DI, HALF], BF16, name="dbu", tag="dbu")
                    veng.tensor_tensor(out=dbu, in0=dtu_t[:, qsl],
                                       in1=bc_t[:, 0:HALF], op=OP.mult)
                    h_t = nwork.tile([DI, HALF], BF16, name="h_t", tag="h_t")
                    init = 0.0 if q == 0 else carry_t[:, n:n + 1]
                    veng.tensor_tensor_scan(
                        out=h_t, data0=en, data1=dbu,
                        initial=init, op0=OP.mult, op1=OP.add)
                    if q == 0:
                        veng.tensor_copy(out=carry_t[:, n:n + 1],
                                         in_=h_t[:, HALF - 1:HALF])
                    hc = nwork.tile([DI, HALF], BF16, name="hc", tag="hc")
                    veng.tensor_tensor(out=hc, in0=h_t,
                                       in1=bc_t[:, HALF:2 * HALF],
                                       op=OP.mult)
                    for k in range(HALF // TC):
                        nc.tensor.matmul(yps[k], lhsT=ident,
                                         rhs=hc[:, k * TC:(k + 1) * TC],
                                         start=(n == 0), stop=(n == DS - 1))
                for k in range(HALF // TC):
                    cs = slice(hs + k * TC, hs + (k + 1) * TC)
                    ks = slice(k * TC, (k + 1) * TC)
                    tmp = small.tile([DI, TC], BF16, name="ytmp", tag="ytmp")
                    nc.vector.scalar_tensor_tensor(
                        out=tmp, in0=u_t[:, cs], scalar=w["D"][:, :],
                        in1=yps[k], op0=OP.mult, op1=OP.add)
                    nc.vector.tensor_tensor(out=yo_h[:, ks], in0=tmp,
                                            in1=zs_t[:, cs], op=OP.mult)

        def post_half(ui, h2):
            s, m = units[ui]
            w = blk[m]
            f2x = feat2x[s]
            yo_h = UST[ui][f"yo{h2}"]
            with nc.named_scope(f"s{s}m{m}_post{h2}"):
                fchsq = postw.tile([2 * DM, HALF], BF16, name="fchsq",
                                   tag="fchsq")
                # mean rows 0:4, mean-square rows 32:36 (PSUM reads must
                # start at a 32-partition boundary)
                ps8 = psB.tile([36, TC], F32, name="ps8", tag="ps8")
                for cc in range(4):
                    ls = slice(cc * TC, (cc + 1) * TC)
                    fps = psA.tile([DM, TC], F32, name="fps", tag="mm")
                    nc.tensor.matmul(fps, lhsT=w["opwT"], rhs=yo_h[:, ls],
                                     start=True, stop=True)
                    # square on GpSimd (from the SBUF copy; GpSimd cannot
                    # read PSUM): halves the post load on the in-order ACT
                    nc.scalar.activation(out=fchsq[0:DM, ls], in_=fps,
                                         func=AF.Copy)
                    nc.gpsimd.tensor_tensor(out=fchsq[DM:2 * DM, ls],
                                            in0=fchsq[0:DM, ls],
                                            in1=fchsq[0:DM, ls], op=OP.mult)
                    nc.tensor.matmul(ps8, lhsT=wsel8[:, cc * 36:(cc + 1) * 36],
                                     rhs=fchsq[:, ls],
                                     start=(cc == 0), stop=(cc == 3))
                # stats rows 0:4 = mu, rows 32:36 = rstd
                stats = postw.tile([36, TC], F32, name="stats", tag="st8")
                nc.scalar.activation(out=stats[0:4, :], in_=ps8[0:4, :],
                                     func=AF.Copy)
                mu2 = stp.tile([4, TC], F32, name="mu2", tag="mu2")
                nc.vector.tensor_tensor(out=mu2, in0=stats[0:4, :],
                                        in1=stats[0:4, :], op=OP.mult)
                var4 = stp.tile([4, TC], F32, name="var4", tag="var")
                nc.vector.tensor_tensor(out=var4, in0=ps8[32:36, :], in1=mu2,
                                        op=OP.subtract)
                # rstd = (var+eps)^-1/2 = exp(-0.5*ln(var+eps)); Ln and Exp
                # coexist in one ACT table (no Sqrt table switch)
                lnv = stp.tile([4, TC], F32, name="lnv", tag="lnv")
                nc.scalar.activation(out=lnv, in_=var4, func=AF.Ln,
                                     bias=eps8[0:4, :])
                nc.scalar.activation(out=stats[32:36, :], in_=lnv,
                                     func=AF.Exp, scale=-0.5)
                for cc in range(4):
                    c = h2 * 4 + cc
                    cs = slice(c * TC, (c + 1) * TC)
                    ls = slice(cc * TC, (cc + 1) * TC)
                    mrbc = psA.tile([2 * DM, TC], F32, name="mrbc", tag="mm")
                    nc.tensor.matmul(mrbc,
                                     lhsT=oh8[:, cc * 128:(cc + 1) * 128],
                                     rhs=stats, start=True, stop=True)
                    t1 = small.tile([DM, TC], BF16, name="t1", tag="t1")
                    nc.vector.tensor_tensor(out=t1, in0=fchsq[0:DM, ls],
                                            in1=mrbc[0:DM, :], op=OP.subtract)
                    t2 = small.tile([DM, TC], BF16, name="t2", tag="t2")
                    nc.vector.tensor_tensor(out=t2, in0=t1,
                                            in1=mrbc[DM:2 * DM, :], op=OP.mult)
                    if m == 0:
                        nc.scalar.activation(
                            out=f2x[0:DM, 3 + c * TC:3 + (c + 1) * TC],
                            in_=t2, func=AF.Identity,
                            scale=w["ln_g"][:, :], bias=w["ln_b"][:, :])
                        nc.scalar.activation(
                            out=f2x[DM:2 * DM, 2 + c * TC:2 + (c + 1) * TC],
                            in_=t2, func=AF.Identity,
                            scale=w["ln_g"][:, :], bias=w["ln_b"][:, :])
                    else:
                        # head input staged in a temp tile so the next
                        # sample's embed never waits on feat2x
                        hd = small.tile([DM, TC], BF16, name="hd", tag="hd")
                        nc.scalar.activation(
                            out=hd, in_=t2, func=AF.Identity,
                            scale=w["ln_g"][:, :], bias=w["ln_b"][:, :])
                        dps = psA.tile([C, TC], F32, name="dps", tag="mm")
                        nc.tensor.matmul(dps, lhsT=sb_headT, rhs=hd,
                                         start=True, stop=True)
                        nd = small.tile([C, TC], BF16, name="nd", tag="nd")
                        nc.scalar.activation(out=nd, in_=dps,
                                             func=AF.Identity,
                                             scale=-1.0, bias=sb_nhb[:, :])
                        zch2 = small.tile([C, TC], F32, name="zch2",
                                          tag="zch")
                        nc.scalar.dma_start(out=zch2, in_=zc[s][:, cs])
                        oc = small.tile([C, TC], F32, name="oc", tag="zch")
                        nc.vector.tensor_tensor(out=oc, in0=zch2, in1=nd,
                                                op=OP.add)
                        nc.gpsimd.dma_start(out=out[s][:, cs], in_=oc)

        # ---- software-pipelined emission ----
        emit_embed(0)
        proj_full(0)
        for k in range(NU):
            scan_half(k, 0)
            if k > 0:
                post_half(k - 1, 1)
            if k == 0:
                emit_embed(1)
            if k + 1 < NU:
                proj_full(k + 1)
            post_half(k, 0)
            scan_half(k, 1)
        post_half(NU - 1, 1)

    nc.finalize()
    return nc


def _prep_maps(inputs):
    import ml_dtypes
    bf = ml_dtypes.bfloat16
    f = np.float32
    z = np.asarray(inputs["z_damaged"], dtype=f).reshape(B, C, L)

    # oh8: per-cc [36,128] broadcast weights: out rows 0:64 get mu[cc],
    # rows 64:128 get rstd[cc]  (stats rows 0:4 = mu, 32:36 = rstd)
    oh8 = np.zeros((36, 4 * 128), f)
    for cc in range(4):
        oh8[cc, cc * 128:cc * 128 + 64] = 1.0
        oh8[32 + cc, cc * 128 + 64:cc * 128 + 128] = 1.0
    # wsel8: per-cc [128,36]: col cc sums rows 0:64 (x)/64, col 32+cc
    # sums rows 64:128 (x^2)/64 (msq lands at PSUM partition 32)
    wsel8 = np.zeros((128, 4 * 36), f)
    for cc in range(4):
        wsel8[0:64, cc * 36 + cc] = 1.0 / DM
        wsel8[64:128, cc * 36 + 32 + cc] = 1.0 / DM

    base = {
        "ident": np.eye(128, dtype=bf),
        "emb_wT": np.ascontiguousarray(np.asarray(inputs["emb_w"], f).T),
        "emb_b": np.asarray(inputs["emb_b"], f).reshape(DM, 1),
        "head_wT": np.ascontiguousarray(np.asarray(inputs["head_w"], f).T).astype(bf),
        "neg_head_b": (-np.asarray(inputs["head_b"], f)).reshape(C, 1),
        "oh8": oh8,
        "wsel8": wsel8.astype(bf),
    }
    for m in (1, 2):
        p = f"m{m}_"
        inw = np.asarray(inputs[p + "in_proj_w"], f)  # [2DI, DM]
        w_u = inw[:DI]  # [DI, DM]
        cw = np.asarray(inputs[p + "conv_w"], f).reshape(DI, DK)
        # lhsT rows (k,m) -> cols d: w[d,k]*W_u[d,m]
        base[p + "cwu0"] = np.ascontiguousarray(np.concatenate(
            [cw[:, 0][None, :] * w_u.T, cw[:, 1][None, :] * w_u.T], axis=0)).astype(bf)
        base[p + "cwu1"] = np.ascontiguousarray(np.concatenate(
            [cw[:, 2][None, :] * w_u.T, cw[:, 3][None, :] * w_u.T], axis=0)).astype(bf)
        base[p + "inw_zT"] = np.ascontiguousarray(inw[DI:].T).astype(bf)
        base[p + "conv_b"] = np.asarray(inputs[p + "conv_b"], f).reshape(DI, 1)
        xpw = np.asarray(inputs[p + "x_proj_w"], f)  # rows: dt(4), B(16), C(16)
        xpw = np.concatenate([xpw[DR:], xpw[:DR]], axis=0)  # -> B, C, dt
        base[p + "xpwT"] = np.ascontiguousarray(xpw.T).astype(bf)
        base[p + "dtpwT"] = np.ascontiguousarray(
            np.asarray(inputs[p + "dt_proj_w"], f).T).astype(bf)
        base[p + "dtp_b"] = np.asarray(inputs[p + "dt_proj_b"], f).reshape(DI, 1)
        base[p + "A"] = -np.exp(np.asarray(inputs[p + "A_log"], f))
        base[p + "D"] = np.asarray(inputs[p + "D"], f).reshape(DI, 1)
        base[p + "opwT"] = np.ascontiguousarray(
            np.asarray(inputs[p + "out_proj_w"], f).T).astype(bf)
        base[p + "ln_g"] = np.asarray(inputs[f"ln{m}_g"], f).reshape(DM, 1)
        base[p + "ln_b"] = np.asarray(inputs[f"ln{m}_b"], f).reshape(DM, 1)

    maps = []
    for k in range(NCORES):
        mkp = dict(base)
        mkp["zc"] = np.ascontiguousarray(z[k * BPC:(k + 1) * BPC])
        maps.append(mkp)
    return maps


def _run(inputs, trace=False):
    from concourse.bass_utils import run_bass_kernel_spmd
    if "nc" not in _CACHE:
        _CACHE["nc"] = _build_program()
    nc = _CACHE["nc"]
    maps = _prep_maps(inputs)
    res = run_bass_kernel_spmd(nc, maps, core_ids=list(range(NCORES)), trace=trace)
    outs = [r["out"] for r in res.results]
    full = np.concatenate(outs, axis=0).reshape(B, C, H, W)
    return full, res


def kernel(**inputs):
    full, _ = _run(inputs, trace=False)
    return full
